# revision 1
# baseline (speedup 1.0000x reference)
"""GPT forward pass on 8 Trainium2 NeuronCores.

Sharding: token-parallel trunk. Core c owns q-tile c (rows 128c..128c+127)
of each of the 4 sequences (512 tokens/core). Attention needs all K/V, which
is AllGathered (bf16) across the 8 cores once per layer. The lm_head is
vocab-sharded (4000 cols/core) over an AllGather of the final hidden states.
All matmuls run in bf16 with fp32 PSUM accumulation; layernorm/softmax
statistics and residual stream stay fp32.

Softmax is computed in transposed layout: S^T[k,q] = (K^T).T @ Q^T, exp on
ScalarE, multiplicative causal mask on VectorE, and the denominators come for
free from the P@V matmul by appending a ones-column to V.
"""

import os
import sys

for _p in ("/opt/trn_rl_repo",):
    if os.path.isdir(_p) and _p not in sys.path:
        sys.path.insert(0, _p)

import numpy as np
import ml_dtypes

BF16NP = ml_dtypes.bfloat16

import concourse.bass as bass
import concourse.mybir as mybir
import concourse.tile as tile
from concourse import bacc
from concourse.bass_utils import run_bass_kernel_spmd
from concourse.masks import make_identity

F32 = mybir.dt.float32
BF = mybir.dt.bfloat16
AF = mybir.ActivationFunctionType

V, C, T, H, L, B = 32000, 1024, 1024, 16, 4, 4
HD = C // H          # 64
FF = 4 * C           # 4096
NCORES = 8
TL = 512             # local tokens per core (4 seqs x 128)
SEQ = B              # 4
NT = TL // 128       # 4  local t-tiles; tile tt holds seq tt rows
NCT = C // 128       # 8  c-tiles
NFT = FF // 128      # 32 f-tiles
VSH = V // NCORES    # 4000 vocab shard
NVC = 8
VCW = VSH // NVC     # 500
LN_EPS = 1e-5

KV_K = C * TL        # elems in K^T region of kv_loc
KV_SZ = 2 * C * TL   # elems per-core kv payload (K^T + V)

_prog_cache = {}


def _ap(t, offset, pattern):
    return bass.AP(tensor=t.tensor if isinstance(t, bass.AP) else t, offset=offset, ap=pattern)


def _build(LL=L, debug=False, sim=False):
    key = (LL, debug, sim)
    if key in _prog_cache:
        return _prog_cache[key]

    nc = bacc.Bacc("TRN2", target_bir_lowering=False, debug=False, num_devices=NCORES)

    x0 = nc.dram_tensor("x0", [TL, C], F32, kind="ExternalInput")
    maskT_d = nc.dram_tensor("maskT", [128, NCT, 128], BF, kind="ExternalInput")
    wq_d = nc.dram_tensor("wq", [L, C, C], BF, kind="ExternalInput")
    wk_d = nc.dram_tensor("wk", [L, C, C], BF, kind="ExternalInput")
    wv_d = nc.dram_tensor("wv", [L, C, C], BF, kind="ExternalInput")
    wo_d = nc.dram_tensor("wo", [L, C, C], BF, kind="ExternalInput")
    w1_d = nc.dram_tensor("w1", [L, C, FF], BF, kind="ExternalInput")
    w2_d = nc.dram_tensor("w2", [L, FF, C], BF, kind="ExternalInput")
    b1_d = nc.dram_tensor("b1", [L, FF], F32, kind="ExternalInput")
    bo_d = nc.dram_tensor("bo", [L, C], F32, kind="ExternalInput")
    b2_d = nc.dram_tensor("b2", [L, C], F32, kind="ExternalInput")
    ln1g_d = nc.dram_tensor("ln1g", [L, C], F32, kind="ExternalInput")
    ln1b_d = nc.dram_tensor("ln1b", [L, C], F32, kind="ExternalInput")
    ln2g_d = nc.dram_tensor("ln2g", [L, C], F32, kind="ExternalInput")
    ln2b_d = nc.dram_tensor("ln2b", [L, C], F32, kind="ExternalInput")
    lnfg_d = nc.dram_tensor("lnfg", [C], F32, kind="ExternalInput")
    lnfb_d = nc.dram_tensor("lnfb", [C], F32, kind="ExternalInput")
    wlm_d = nc.dram_tensor("wlm", [C, VSH], BF, kind="ExternalInput")
    blm_d = nc.dram_tensor("blm", [VSH], BF, kind="ExternalInput")

    logits_d = nc.dram_tensor("logits", [NCORES * TL, VSH], F32, kind="ExternalOutput")
    dbg_d = None
    if debug:
        dbg_d = nc.dram_tensor("dbg", [LL, TL, C], F32, kind="ExternalOutput")

    with tile.TileContext(nc) as tc:
        import contextlib

        with contextlib.ExitStack() as ctx:
            # SBUF pools (per-partition KB in comments)
            const = ctx.enter_context(tc.tile_pool(name="const", bufs=1))      # ~2.5
            xpool = ctx.enter_context(tc.tile_pool(name="x", bufs=1))          # 16
            hpool = ctx.enter_context(tc.tile_pool(name="h", bufs=5))          # 10
            tpool = ctx.enter_context(tc.tile_pool(name="hT", bufs=1))         # 8
            qtpool = ctx.enter_context(tc.tile_pool(name="qt", bufs=1))        # 8
            kvpool = ctx.enter_context(tc.tile_pool(name="kv", bufs=4))        # 4
            otpool = ctx.enter_context(tc.tile_pool(name="oT", bufs=1))        # 8
            big = ctx.enter_context(tc.tile_pool(name="big", bufs=2))          # 64
            wpool = ctx.enter_context(tc.tile_pool(name="w", bufs=6))          # 6
            gbpool = ctx.enter_context(tc.tile_pool(name="gb", bufs=1))        # 16
            misc = ctx.enter_context(tc.tile_pool(name="misc", bufs=2))        # ~1
            lntmp = ctx.enter_context(tc.tile_pool(name="lntmp", bufs=2))      # 8
            kts_pool = ctx.enter_context(tc.tile_pool(name="kts", bufs=3))     # 4
            pt_pool = ctx.enter_context(tc.tile_pool(name="pt", bufs=3))       # 6
            oraw_pool = ctx.enter_context(tc.tile_pool(name="oraw", bufs=2))   # 20
            rcp_pool = ctx.enter_context(tc.tile_pool(name="rcp", bufs=1))     # 8
            rb_pool = ctx.enter_context(tc.tile_pool(name="rb", bufs=1))       # 16
            lmh_pool = ctx.enter_context(tc.tile_pool(name="lmh", bufs=1))     # 8
            lgout = ctx.enter_context(tc.tile_pool(name="lgout", bufs=2))      # 4
            lgb_pool = ctx.enter_context(tc.tile_pool(name="lgb", bufs=1))     # 8
            ps_acc = ctx.enter_context(tc.tile_pool(name="psacc", bufs=5, space="PSUM"))
            ps_st = ctx.enter_context(tc.tile_pool(name="psst", bufs=2, space="PSUM"))
            ps_ov = ctx.enter_context(tc.tile_pool(name="psov", bufs=1, space="PSUM"))
            dram = ctx.enter_context(tc.tile_pool(name="dram", bufs=1, space="DRAM"))

            ident = const.tile([128, 128], BF, name="ident")
            make_identity(nc, ident)
            eps_t = const.tile([128, 1], F32, name="eps")
            nc.vector.memset(eps_t[:], LN_EPS)
            maskT = const.tile([128, NCT, 128], BF, name="maskT")
            nc.sync.dma_start(out=maskT[:], in_=maskT_d[:])

            kv_loc = dram.tile([KV_SZ], BF, name="kv_loc")
            hfT_loc = dram.tile([C * TL], BF, name="hfT_loc")
            hfT_full = dram.tile([NCORES * C * TL], BF, addr_space="Local" if sim else "Shared", name="hfT_full")
            rc_bounce = dram.tile([SEQ, H * 128], F32, name="rc_bounce")

            # persistent residual stream, fp32: tile tt = seq tt, partition j
            x_t = [xpool.tile([128, C], F32, tag=f"x{tt}", name=f"x{tt}") for tt in range(NT)]
            for tt in range(NT):
                nc.sync.dma_start(out=x_t[tt][:], in_=x0[tt * 128:(tt + 1) * 128, :])

            def bcast_row(dst, src_1d_tensor, offset, n):
                # replicate a [n] dram row across all partitions of dst [P, n]
                src = _ap(src_1d_tensor, offset, [[0, dst.shape[0]], [1, n]])
                nc.gpsimd.dma_start(out=dst[:], in_=src)

            def emit_ln(g_t, b_t):
                """LayerNorm over free dim of x_t -> transposed bf16 hT tiles."""
                h_tiles = []
                for tt in range(NT):
                    stats = misc.tile([128, 2, 6], F32, name="stats", tag="stats")
                    xv = x_t[tt][:].rearrange("p (s d) -> p s d", s=2)
                    nc.vector.bn_stats(out=stats[:, 0, :], in_=xv[:, 0, :])
                    nc.vector.bn_stats(out=stats[:, 1, :], in_=xv[:, 1, :])
                    mv = misc.tile([128, 2], F32, name="mv", tag="mv")
                    nc.vector.bn_aggr(out=mv[:], in_=stats[:])
                    rstd = misc.tile([128, 1], F32, name="rstd", tag="rstd")
                    nc.scalar.activation(rstd[:], mv[:, 1:2], AF.Sqrt, bias=eps_t[:])
                    nc.vector.reciprocal(rstd[:], rstd[:])
                    xn = lntmp.tile([128, C], F32, tag="xn", name="xn")
                    nc.vector.tensor_scalar(
                        out=xn[:], in0=x_t[tt][:], scalar1=mv[:, 0:1], scalar2=rstd[:],
                        op0=mybir.AluOpType.subtract, op1=mybir.AluOpType.mult,
                    )
                    nc.vector.tensor_mul(out=xn[:], in0=xn[:], in1=g_t[:])
                    h = hpool.tile([128, C], BF, tag="h", name="h")
                    nc.vector.tensor_add(out=h[:], in0=xn[:], in1=b_t[:])
                    h_tiles.append(h)
                hT_tiles = []
                for ct in range(NCT):
                    pst = ps_st.tile([128, 512], BF, tag="st", name="pst")
                    for tt in range(NT):
                        nc.tensor.transpose(
                            pst[:, tt * 128:(tt + 1) * 128],
                            h_tiles[tt][:, ct * 128:(ct + 1) * 128],
                            ident[:],
                        )
                    hT = tpool.tile([128, 512], BF, tag=f"hT{ct}", name=f"hT{ct}")
                    nc.vector.tensor_copy(out=hT[:], in_=pst[:])
                    hT_tiles.append(hT)
                return hT_tiles

            def load_w_tile(wd, l_idx, r0, c0, rows=128, cols=512):
                wt = wpool.tile([rows, cols], BF, tag="w", name="wt")
                nc.sync.dma_start(out=wt[:], in_=wd[l_idx, r0:r0 + rows, c0:c0 + cols])
                return wt

            for l in range(LL):
                lw = l % L
                g1 = gbpool.tile([128, C], F32, tag="g", name="g1")
                bcast_row(g1, ln1g_d, lw * C, C)
                bb1 = gbpool.tile([128, C], F32, tag="b", name="bb1")
                bcast_row(bb1, ln1b_d, lw * C, C)
                hT = emit_ln(g1, bb1)

                # ---- QKV projections ----
                # Q^T (resident), K^T (streamed to kv_loc): out[d,t] = sum_c W[c,d] hT[c,t]
                qT = []
                for name, wd in (("q", wq_d), ("k", wk_d)):
                    for dtg in range(2):
                        pss = [ps_acc.tile([128, 512], F32, tag="acc", name="acc") for _ in range(4)]
                        for ct in range(NCT):
                            wt = load_w_tile(wd, lw, ct * 128, dtg * 512)
                            for d4 in range(4):
                                nc.tensor.matmul(
                                    pss[d4][:], wt[:, d4 * 128:(d4 + 1) * 128], hT[ct][:],
                                    start=(ct == 0), stop=(ct == NCT - 1),
                                )
                        for d4 in range(4):
                            dt = dtg * 4 + d4
                            if name == "q":
                                ot = qtpool.tile([128, 512], BF, tag=f"qT{dt}", name=f"qT{dt}")
                                nc.vector.tensor_copy(out=ot[:], in_=pss[d4][:])
                                qT.append(ot)
                            else:
                                ot = kvpool.tile([128, 512], BF, tag="kv", name="kTs_out")
                                nc.vector.tensor_copy(out=ot[:], in_=pss[d4][:])
                                nc.sync.dma_start(
                                    out=_ap(kv_loc, dt * 128 * TL, [[TL, 128], [1, TL]]),
                                    in_=ot[:],
                                )
                # V natural (streamed in halves): out[t,c'] = sum_c hT[c,t] Wv[c,c']
                for nf in range(2):
                    pss = [ps_acc.tile([128, 512], F32, tag="acc", name="acc") for _ in range(4)]
                    for ct in range(NCT):
                        wt = load_w_tile(wv_d, lw, ct * 128, nf * 512)
                        for tt in range(NT):
                            nc.tensor.matmul(
                                pss[tt][:], hT[ct][:, tt * 128:(tt + 1) * 128], wt[:],
                                start=(ct == 0), stop=(ct == NCT - 1),
                            )
                    for tt in range(NT):
                        vt = kvpool.tile([128, 512], BF, tag="kv", name="v_out")
                        nc.vector.tensor_copy(out=vt[:], in_=pss[tt][:])
                        nc.sync.dma_start(
                            out=_ap(kv_loc, KV_K + tt * 128 * C + nf * 512, [[C, 128], [1, 512]]),
                            in_=vt[:],
                        )
                kv_full = dram.tile([NCORES * KV_SZ], BF, addr_space="Local" if sim else "Shared", name=f"kv_full{l}")
                if sim:
                    nc.sync.dma_start(
                        out=_ap(kv_full, 0, [[2048, KV_SZ // 2048], [1, 2048]]),
                        in_=_ap(kv_loc, 0, [[2048, KV_SZ // 2048], [1, 2048]]),
                    )
                else:
                    nc.gpsimd.collective_compute(
                        "AllGather",
                        mybir.AluOpType.bypass,
                        replica_groups=[list(range(NCORES))],
                        ins=[_ap(kv_loc, 0, [[2048, KV_SZ // 2048], [1, 2048]])],
                        outs=[_ap(kv_full, 0, [[2048, NCORES * KV_SZ // 2048], [1, 2048]])],
                    )

                # ---- attention ----
                # O^T as one tile: row c = ct*128 + p, free = (ct, t)
                oT = otpool.tile([128, NCT, 512], BF, tag="oT", name="oT")
                for s in range(SEQ):
                    # V for seq s, all ranks/heads, with a ones column per head:
                    # v_s[j, r, h, 0:64] = V_r[s*128+j, h*64+d]; v_s[..., 64] = 1
                    v_s = big.tile([128, NCT, H, HD + 1], BF, tag="big", name="vs")
                    nc.vector.memset(v_s[:, :, :, HD:HD + 1], 1.0)
                    for r in range(NCORES):
                        nc.sync.dma_start(
                            out=v_s[:, r, :, 0:HD],
                            in_=_ap(
                                kv_full,
                                r * KV_SZ + KV_K + s * 128 * C,
                                [[C, 128], [HD, H], [1, HD]],
                            ),
                        )
                    oraw = oraw_pool.tile([HD + 1, H, 128], F32, tag="oraw", name="oraw")
                    for h in range(H):
                        poff = (h % 2) * HD  # parity offset matches qT slices
                        kTs = kts_pool.tile([128, NCT, 128], BF, tag="kts", name="kts")
                        nc.sync.dma_start(
                            out=kTs[poff:poff + HD, :, :],
                            in_=_ap(
                                kv_full,
                                h * HD * TL + s * 128,
                                [[TL, HD], [KV_SZ, NCORES], [1, 128]],
                            ),
                        )
                        q_sl = qT[h // 2][poff:poff + HD, s * 128:(s + 1) * 128]
                        pT = pt_pool.tile([128, NCT, 128], BF, tag="pt", name="pt")
                        for half in range(2):
                            st = ps_st.tile([128, 4, 128], F32, tag="st", name="st")
                            for k4 in range(4):
                                nc.tensor.matmul(
                                    st[:, k4, :], kTs[poff:poff + HD, half * 4 + k4, :], q_sl,
                                    start=True, stop=True,
                                )
                            nc.scalar.activation(
                                pT[:, half * 4:half * 4 + 4, :], st[:], AF.Exp
                            )
                        nc.vector.tensor_mul(out=pT[:], in0=pT[:], in1=maskT[:])
                        ov = ps_ov.tile([128, 128], F32, tag="ov", name="ov")
                        for kt in range(NCT):
                            nc.tensor.matmul(
                                ov[0:HD + 1, :], v_s[:, kt, h, :], pT[:, kt, :],
                                start=(kt == 0), stop=(kt == NCT - 1),
                            )
                        nc.vector.tensor_copy(out=oraw[:, h, :], in_=ov[0:HD + 1, :])
                    # denominators -> reciprocal -> broadcast over 64 partitions
                    recips = rcp_pool.tile([1, H, 128], F32, tag="recips", name="recips")
                    nc.vector.reciprocal(recips[:], oraw[HD:HD + 1, :, :])
                    nc.sync.dma_start(out=rc_bounce[s, :], in_=recips[:])
                    rb = rb_pool.tile([HD, H, 128], F32, tag="rb", name="rb")
                    nc.gpsimd.dma_start(
                        out=rb[:], in_=_ap(rc_bounce, s * H * 128, [[0, HD], [128, H], [1, 128]])
                    )
                    # even heads: normalize straight into oT (partitions 0-63);
                    # odd heads: stage then DMA into partitions 64-127
                    oS = oraw_pool.tile([HD, NCT, 128], BF, tag="oS", name="oS")
                    for h in range(H):
                        if h % 2 == 0:
                            dst = oT[0:HD, h // 2, s * 128:(s + 1) * 128]
                        else:
                            dst = oS[:, h // 2, :]
                        nc.vector.tensor_mul(out=dst, in0=oraw[0:HD, h, :], in1=rb[:, h, :])
                    nc.sync.dma_start(
                        out=oT[HD:128, :, s * 128:(s + 1) * 128], in_=oS[:],
                    )

                # ---- output projection + residual ----
                bo_t = gbpool.tile([128, C], F32, tag="b", name="bo_t")
                bcast_row(bo_t, bo_d, lw * C, C)
                for nf in range(2):
                    pss = [ps_acc.tile([128, 512], F32, tag="acc", name="acc") for _ in range(4)]
                    for ct in range(NCT):
                        wt = load_w_tile(wo_d, lw, ct * 128, nf * 512)
                        for tt in range(NT):
                            nc.tensor.matmul(
                                pss[tt][:], oT[:, ct, tt * 128:(tt + 1) * 128], wt[:],
                                start=(ct == 0), stop=(ct == NCT - 1),
                            )
                    for tt in range(NT):
                        xs = x_t[tt][:, nf * 512:(nf + 1) * 512]
                        nc.vector.tensor_add(out=xs, in0=xs, in1=pss[tt][:])
                        nc.vector.tensor_add(out=xs, in0=xs, in1=bo_t[:, nf * 512:(nf + 1) * 512])

                # ---- FFN ----
                g2 = gbpool.tile([128, C], F32, tag="g", name="g2")
                bcast_row(g2, ln2g_d, lw * C, C)
                bb2 = gbpool.tile([128, C], F32, tag="b", name="bb2")
                bcast_row(bb2, ln2b_d, lw * C, C)
                h2T = emit_ln(g2, bb2)

                b1_t = misc.tile([128, NFT], F32, tag="b1", name="b1_t")
                nc.gpsimd.dma_start(
                    out=b1_t[:], in_=_ap(b1_d, lw * FF, [[1, 128], [128, NFT]])
                )
                ug = big.tile([128, NFT, 512], BF, tag="big", name="ug")
                for fg in range(8):
                    pss = [ps_acc.tile([128, 512], F32, tag="acc", name="acc") for _ in range(4)]
                    for ct in range(NCT):
                        wt = load_w_tile(w1_d, lw, ct * 128, fg * 512)
                        for f4 in range(4):
                            nc.tensor.matmul(
                                pss[f4][:], wt[:, f4 * 128:(f4 + 1) * 128], h2T[ct][:],
                                start=(ct == 0), stop=(ct == NCT - 1),
                            )
                    for f4 in range(4):
                        ft = fg * 4 + f4
                        nc.scalar.activation(
                            ug[:, ft, :], pss[f4][:], AF.Gelu, bias=b1_t[:, ft:ft + 1]
                        )

                b2_t = gbpool.tile([128, C], F32, tag="b", name="b2_t")
                bcast_row(b2_t, b2_d, lw * C, C)
                for nf in range(2):
                    pss = [ps_acc.tile([128, 512], F32, tag="acc", name="acc") for _ in range(4)]
                    for ft in range(NFT):
                        wt = load_w_tile(w2_d, lw, ft * 128, nf * 512)
                        for tt in range(NT):
                            nc.tensor.matmul(
                                pss[tt][:], ug[:, ft, tt * 128:(tt + 1) * 128], wt[:],
                                start=(ft == 0), stop=(ft == NFT - 1),
                            )
                    for tt in range(NT):
                        xs = x_t[tt][:, nf * 512:(nf + 1) * 512]
                        nc.vector.tensor_add(out=xs, in0=xs, in1=pss[tt][:])
                        nc.vector.tensor_add(out=xs, in0=xs, in1=b2_t[:, nf * 512:(nf + 1) * 512])

                if debug:
                    for tt in range(NT):
                        nc.sync.dma_start(
                            out=dbg_d[l, tt * 128:(tt + 1) * 128, :], in_=x_t[tt][:]
                        )

            # ---- final LN, AllGather h_f^T, lm_head ----
            gf = gbpool.tile([128, C], F32, tag="g", name="gf")
            bcast_row(gf, lnfg_d, 0, C)
            bft = gbpool.tile([128, C], F32, tag="b", name="bft")
            bcast_row(bft, lnfb_d, 0, C)
            hfT = emit_ln(gf, bft)
            for ct in range(NCT):
                nc.sync.dma_start(
                    out=_ap(hfT_loc, ct * 128 * TL, [[TL, 128], [1, TL]]),
                    in_=hfT[ct][:],
                )
            if sim:
                nc.sync.dma_start(
                    out=_ap(hfT_full, 0, [[2048, C * TL // 2048], [1, 2048]]),
                    in_=_ap(hfT_loc, 0, [[2048, C * TL // 2048], [1, 2048]]),
                )
            else:
                nc.gpsimd.collective_compute(
                    "AllGather",
                    mybir.AluOpType.bypass,
                    replica_groups=[list(range(NCORES))],
                    ins=[_ap(hfT_loc, 0, [[2048, C * TL // 2048], [1, 2048]])],
                    outs=[_ap(hfT_full, 0, [[2048, NCORES * C * TL // 2048], [1, 2048]])],
                )

            # lm_head in vocab halves: wlm half resident, hf streamed per r
            for vq in range(2):
                wlm_q = big.tile([128, NCT, 4 * VCW], BF, tag="big", name="wlmq")
                for ct in range(NCT):
                    nc.sync.dma_start(
                        out=wlm_q[:, ct, :],
                        in_=wlm_d[ct * 128:(ct + 1) * 128, vq * 4 * VCW:(vq + 1) * 4 * VCW],
                    )
                blm_qs = []
                for v4 in range(4):
                    bq = lgb_pool.tile([128, VCW], BF, tag="lgb", name="blmq", bufs=4)
                    bcast_row(bq, blm_d, (vq * 4 + v4) * VCW, VCW)
                    blm_qs.append(bq)
                for r in range(NCORES):
                    hfr = lmh_pool.tile([128, NCT, 512], BF, tag="hfr", name="hfr")
                    for ct in range(NCT):
                        nc.sync.dma_start(
                            out=hfr[:, ct, :],
                            in_=_ap(hfT_full, r * C * TL + ct * 128 * TL, [[TL, 128], [1, TL]]),
                        )
                    for ts in range(NT):
                        pss = [ps_acc.tile([128, VCW], F32, tag="acc", name="acc") for _ in range(4)]
                        for ct in range(NCT):
                            for v4 in range(4):
                                nc.tensor.matmul(
                                    pss[v4][:],
                                    hfr[:, ct, ts * 128:(ts + 1) * 128],
                                    wlm_q[:, ct, v4 * VCW:(v4 + 1) * VCW],
                                    start=(ct == 0), stop=(ct == NCT - 1),
                                )
                        for v4 in range(4):
                            vc = vq * 4 + v4
                            lg = lgout.tile([128, VCW], F32, tag="lg", name="lg")
                            nc.vector.tensor_add(
                                out=lg[:], in0=pss[v4][:],
                                in1=blm_qs[v4][:],
                            )
                            row0 = r * TL + ts * 128
                            nc.sync.dma_start(
                                out=logits_d[row0:row0 + 128, vc * VCW:(vc + 1) * VCW],
                                in_=lg[:],
                            )

    nc.compile()
    _prog_cache[key] = nc
    return nc


def _prep_inputs(inputs):
    f = {k: np.asarray(v) for k, v in inputs.items()}
    idx = f["idx"].astype(np.int64)
    emb = f["emb"].astype(np.float32)
    pos = f["pos_enc"].astype(np.float32)
    x_full = emb[idx] + pos[None, :, :]          # [B, T, C] f32

    scale = HD ** -0.5
    bf = lambda a: np.ascontiguousarray(a, dtype=np.float32).astype(BF16NP)
    shared = {
        "wq": bf(f["Wq"] * scale),
        "wk": bf(f["Wk"]),
        "wv": bf(f["Wv"]),
        "wo": bf(f["Wo"]),
        "w1": bf(f["W1"]),
        "w2": bf(f["W2"]),
        "b1": f["b1"].astype(np.float32),
        "bo": f["bo"].astype(np.float32),
        "b2": f["b2"].astype(np.float32),
        "ln1g": f["ln1_g"].astype(np.float32),
        "ln1b": f["ln1_b"].astype(np.float32),
        "ln2g": f["ln2_g"].astype(np.float32),
        "ln2b": f["ln2_b"].astype(np.float32),
        "lnfg": f["lnf_g"].astype(np.float32),
        "lnfb": f["lnf_b"].astype(np.float32),
    }
    wlm_f = f["Wlm"].astype(np.float32)
    blm_f = f["blm"].astype(np.float32)

    in_maps = []
    kk = np.arange(T)[:, None]
    for c in range(NCORES):
        x0_c = np.ascontiguousarray(
            x_full[:, 128 * c:128 * (c + 1), :].reshape(TL, C), dtype=np.float32
        )
        jj = np.arange(128)[None, :]
        m = (kk <= 128 * c + jj).astype(np.float32)      # [T, 128]
        maskT_c = np.ascontiguousarray(
            m.reshape(NCT, 128, 128).transpose(1, 0, 2)
        ).astype(BF16NP)                                  # [128(kk), 8(kt), 128(j)]
        im = dict(shared)
        im["x0"] = x0_c
        im["maskT"] = maskT_c
        im["wlm"] = np.ascontiguousarray(wlm_f[:, c * VSH:(c + 1) * VSH]).astype(BF16NP)
        im["blm"] = np.ascontiguousarray(blm_f[c * VSH:(c + 1) * VSH]).astype(BF16NP)
        in_maps.append(im)
    return in_maps


def kernel(**inputs):
    nc = _build()
    in_maps = _prep_inputs(inputs)
    res = run_bass_kernel_spmd(nc, in_maps, list(range(NCORES)))
    # per-core logits rows are [r(8), s(4), j(128)]; vocab sharded on cores
    parts = [r["logits"].reshape(NCORES, SEQ, 128, VSH) for r in res.results]
    full = np.concatenate(parts, axis=-1)                 # [r, s, j, V]
    full = full.transpose(1, 0, 2, 3).reshape(B, T, V)    # [s, r*128+j, V]
    return np.ascontiguousarray(full, dtype=np.float32)



# revision 8
# speedup vs baseline: 1.1278x; 1.1278x over previous
"""GPT forward pass on 8 Trainium2 NeuronCores — v2.

Sharding:
  - Residual/trunk GEMMs (Wo, FFN) are token-parallel: core c owns q-tile c
    (rows 128c..128c+127) of each of the 4 sequences (512 tokens/core).
  - QKV projection + attention are head-sharded: core c owns heads {c, c+8}
    over ALL 4096 tokens. Per layer the post-LN hidden states h^T are
    AllGathered (1 MB/rank, bf16); each core computes Q/K/V for its two heads
    over all tokens and runs full causal attention for its 8 (seq, head)
    pairs, skipping above-diagonal 128x128 tiles. Attention outputs return to
    token owners via AllToAll (1 MB/rank), pre-chunked by consumer.
  - lm_head is vocab-sharded (4000 cols/core, padded to 4096).

LayerNorm gains/biases are folded into weights host-side (g into W rows, b
into per-output-channel biases), so on-chip LN is just (x-mean)*rstd. The
V-path bias is folded into bo via bo' = bo + (b@Wv)@Wo. Softmax denominators
come free from a ones-column appended to V; exp runs on ScalarE over causal
tiles only. All matmuls bf16 with fp32 PSUM; residual stream fp32.
"""

import os
import sys

for _p in ("/opt/trn_rl_repo",):
    if os.path.isdir(_p) and _p not in sys.path:
        sys.path.insert(0, _p)

import numpy as np
import ml_dtypes

BF16NP = ml_dtypes.bfloat16

import concourse.bass as bass
import concourse.mybir as mybir
import concourse.tile as tile
from concourse import bacc
from concourse.bass_utils import run_bass_kernel_spmd
from concourse.masks import make_identity

F32 = mybir.dt.float32
BF = mybir.dt.bfloat16
AF = mybir.ActivationFunctionType

V, C, T, H, L, B = 32000, 1024, 1024, 16, 4, 4
HD = C // H          # 64
FF = 4 * C           # 4096
NCORES = 8
TL = 512             # local tokens per core (4 seqs x 128)
SEQ = B              # 4
NT = TL // 128       # 4
NCT = C // 128       # 8
VSH = V // NCORES    # 4000
VPAD = 4096          # padded vocab shard
LN_EPS = 1e-5

HT_SZ = C * TL            # one rank's h^T payload elems (512K)
O_SZ = NCORES * 128 * TL  # o payload elems per rank (8 consumer chunks)

# packed causal slot table: slot(kt, qt) defined for kt <= qt
_SLOT = {}
_off = 0
for _kt in range(8):
    for _qt in range(_kt, 8):
        _SLOT[(_kt, _qt)] = _off
        _off += 1
NSLOT = _off  # 36

_prog_cache = {}


def _ap(t, offset, pattern):
    return bass.AP(tensor=t.tensor if isinstance(t, bass.AP) else t, offset=offset, ap=pattern)


def _build(LL=L, debug=False, sim=False):
    key = (LL, debug, sim)
    if key in _prog_cache:
        return _prog_cache[key]

    nc = bacc.Bacc("TRN2", target_bir_lowering=False, debug=False, num_devices=NCORES)

    x0_d = nc.dram_tensor("x0", [TL, C], F32, kind="ExternalInput")
    maskD_d = nc.dram_tensor("maskD", [128, 128], BF, kind="ExternalInput")
    wq_d = nc.dram_tensor("wq", [L, C, 128], BF, kind="ExternalInput")
    wk_d = nc.dram_tensor("wk", [L, C, 128], BF, kind="ExternalInput")
    wv_d = nc.dram_tensor("wv", [L, C, 128], BF, kind="ExternalInput")
    qb_d = nc.dram_tensor("qb", [L, 128], F32, kind="ExternalInput")
    kb_d = nc.dram_tensor("kb", [L, 128], F32, kind="ExternalInput")
    wo_d = nc.dram_tensor("wo", [L, C, C], BF, kind="ExternalInput")
    bo_d = nc.dram_tensor("bo", [L, C], F32, kind="ExternalInput")
    w1_d = nc.dram_tensor("w1", [L, C, FF], BF, kind="ExternalInput")
    b1_d = nc.dram_tensor("b1", [L, FF], F32, kind="ExternalInput")
    w2_d = nc.dram_tensor("w2", [L, FF, C], BF, kind="ExternalInput")
    b2_d = nc.dram_tensor("b2", [L, C], F32, kind="ExternalInput")
    # wlm layout: [vg 4][ct 8][cp 128][vt 8 * 128 v]
    wlm_d = nc.dram_tensor("wlm", [4, NCT, 128, 1024], BF, kind="ExternalInput")
    blm_d = nc.dram_tensor("blm", [VPAD], F32, kind="ExternalInput")

    logits_d = nc.dram_tensor("logits", [VPAD, NCORES * TL], F32, kind="ExternalOutput")
    dbg_d = None
    if debug:
        dbg_d = nc.dram_tensor("dbg", [LL, TL, C], F32, kind="ExternalOutput")
        dbgq_d = nc.dram_tensor("dbgq", [128, NCORES, TL], F32, kind="ExternalOutput")
        dbgk_d = nc.dram_tensor("dbgk", [128, NCORES, TL], F32, kind="ExternalOutput")
        dbgv_d = nc.dram_tensor("dbgv", [128, NCORES, SEQ, 2, HD + 1], F32, kind="ExternalOutput")
        dbgos_d = nc.dram_tensor("dbgos", [128, NCORES, SEQ, 128], F32, kind="ExternalOutput")
        dbgot_d = nc.dram_tensor("dbgot", [128, NCT, TL], F32, kind="ExternalOutput")

    with tile.TileContext(nc) as tc:
        import contextlib

        with contextlib.ExitStack() as ctx:
            # SBUF pools (per-partition KB in comments)
            const = ctx.enter_context(tc.tile_pool(name="const", bufs=1))      # 0.7
            xpool = ctx.enter_context(tc.tile_pool(name="x", bufs=1))          # 16
            hpool = ctx.enter_context(tc.tile_pool(name="h", bufs=1))          # 8
            tpool = ctx.enter_context(tc.tile_pool(name="hT", bufs=1))         # 8
            wqkv_pool = ctx.enter_context(tc.tile_pool(name="wqkv", bufs=1))   # 6
            big8 = ctx.enter_context(tc.tile_pool(name="big8", bufs=3))        # 24
            qkpool = ctx.enter_context(tc.tile_pool(name="qk", bufs=1))        # 16
            vpool = ctx.enter_context(tc.tile_pool(name="v", bufs=1))          # 8.3
            ptpool = ctx.enter_context(tc.tile_pool(name="pt", bufs=2))        # 18
            orawp = ctx.enter_context(tc.tile_pool(name="oraw", bufs=1))       # 8
            rbp = ctx.enter_context(tc.tile_pool(name="rb", bufs=1))           # 8
            osp = ctx.enter_context(tc.tile_pool(name="oS", bufs=2))           # 4
            otsend = ctx.enter_context(tc.tile_pool(name="otsend", bufs=1))    # 8
            otpool = ctx.enter_context(tc.tile_pool(name="oT", bufs=1))        # 8
            ugpool = ctx.enter_context(tc.tile_pool(name="ug", bufs=1))        # 32
            wlmpool = ctx.enter_context(tc.tile_pool(name="wlm", bufs=1))      # 16
            lgout = ctx.enter_context(tc.tile_pool(name="lgout", bufs=2))      # 4
            gbpool = ctx.enter_context(tc.tile_pool(name="gb", bufs=1))        # 8
            misc = ctx.enter_context(tc.tile_pool(name="misc", bufs=2))        # 1.5
            # PSUM: acc 4x[128,512]f32 (4 banks) + sm 8x[128,128]f32 (2 banks)
            # + tp 2x[128,512]bf16 (1 bank) = 7 of 8 banks
            ps_acc = ctx.enter_context(tc.tile_pool(name="psacc", bufs=4, space="PSUM"))
            ps_sm = ctx.enter_context(tc.tile_pool(name="pssm", bufs=2, space="PSUM"))
            ps_tp = ctx.enter_context(tc.tile_pool(name="pstp", bufs=2, space="PSUM"))
            dram = ctx.enter_context(tc.tile_pool(name="dram", bufs=1, space="DRAM"))

            ident = const.tile([128, 128], BF, name="ident")
            make_identity(nc, ident)
            eps_t = const.tile([128, 1], F32, name="eps")
            nc.vector.memset(eps_t[:], LN_EPS)
            maskD = const.tile([128, 128], BF, name="maskD")
            nc.sync.dma_start(out=maskD[:], in_=maskD_d[:])

            hT_loc = dram.tile([HT_SZ], BF, name="hT_loc")
            o_loc = dram.tile([O_SZ], BF, name="o_loc")
            rc_bounce = dram.tile([SEQ, 2 * NCT * 128], F32, name="rc_bounce")

            x_t = [xpool.tile([128, C], F32, tag=f"x{tt}", name=f"x{tt}") for tt in range(NT)]
            for tt in range(NT):
                nc.sync.dma_start(out=x_t[tt][:], in_=x0_d[tt * 128:(tt + 1) * 128, :])

            def bcast_row(dst, src_1d, offset, n):
                src = _ap(src_1d, offset, [[0, dst.shape[0]], [1, n]])
                nc.gpsimd.dma_start(out=dst[:], in_=src)

            def emit_ln():
                """(x - mean) * rstd -> bf16, transposed into 8 hT tiles."""
                h_tiles = []
                for tt in range(NT):
                    stats = misc.tile([128, 2, 6], F32, name="stats", tag="stats")
                    xv = x_t[tt][:].rearrange("p (s d) -> p s d", s=2)
                    nc.vector.bn_stats(out=stats[:, 0, :], in_=xv[:, 0, :])
                    nc.vector.bn_stats(out=stats[:, 1, :], in_=xv[:, 1, :])
                    mv = misc.tile([128, 2], F32, name="mv", tag="mv")
                    nc.vector.bn_aggr(out=mv[:], in_=stats[:])
                    rstd = misc.tile([128, 1], F32, name="rstd", tag="rstd")
                    nc.scalar.activation(rstd[:], mv[:, 1:2], AF.Sqrt, bias=eps_t[:])
                    nc.vector.reciprocal(rstd[:], rstd[:])
                    h = hpool.tile([128, C], BF, tag=f"h{tt}", name=f"h{tt}")
                    nc.vector.tensor_scalar(
                        out=h[:], in0=x_t[tt][:], scalar1=mv[:, 0:1], scalar2=rstd[:],
                        op0=mybir.AluOpType.subtract, op1=mybir.AluOpType.mult,
                    )
                    h_tiles.append(h)
                hT_tiles = []
                for ct in range(NCT):
                    pst = ps_tp.tile([128, 512], BF, tag="tp", name="pst")
                    for tt in range(NT):
                        nc.tensor.transpose(
                            pst[:, tt * 128:(tt + 1) * 128],
                            h_tiles[tt][:, ct * 128:(ct + 1) * 128],
                            ident[:],
                        )
                    hT = tpool.tile([128, 512], BF, tag=f"hT{ct}", name=f"hT{ct}")
                    nc.vector.tensor_copy(out=hT[:], in_=pst[:])
                    hT_tiles.append(hT)
                return hT_tiles

            def gather_hT(hT_tiles, lname):
                """write local h^T -> DRAM, AllGather -> [r][ct][cp][512]."""
                for ct in range(NCT):
                    nc.sync.dma_start(
                        out=_ap(hT_loc, ct * 128 * TL, [[TL, 128], [1, TL]]),
                        in_=hT_tiles[ct][:],
                    )
                full = dram.tile([NCORES * HT_SZ], BF,
                                 addr_space="Local" if sim else "Shared", name=lname)
                if sim:
                    nc.sync.dma_start(
                        out=_ap(full, 0, [[2048, HT_SZ // 2048], [1, 2048]]),
                        in_=_ap(hT_loc, 0, [[2048, HT_SZ // 2048], [1, 2048]]),
                    )
                else:
                    nc.gpsimd.collective_compute(
                        "AllGather",
                        mybir.AluOpType.bypass,
                        replica_groups=[list(range(NCORES))],
                        ins=[_ap(hT_loc, 0, [[2048, HT_SZ // 2048], [1, 2048]])],
                        outs=[_ap(full, 0, [[2048, NCORES * HT_SZ // 2048], [1, 2048]])],
                    )
                return full

            # v_sb: [128 j, kt 8, s 4, hs 2, 65]; col 64 is the ones column
            v_sb = vpool.tile([128, NCORES, SEQ, 2, HD + 1], BF, name="v_sb")
            nc.vector.memset(v_sb[:, :, :, :, HD:HD + 1], 1.0)

            for l in range(LL):
                lw = l % L
                wq_sb = wqkv_pool.tile([128, NCT, 128], BF, tag="wq", name="wq_sb")
                nc.sync.dma_start(
                    out=wq_sb[:],
                    in_=_ap(wq_d, lw * C * 128, [[128, 128], [128 * 128, NCT], [1, 128]]))
                wk_sb = wqkv_pool.tile([128, NCT, 128], BF, tag="wk", name="wk_sb")
                nc.sync.dma_start(
                    out=wk_sb[:],
                    in_=_ap(wk_d, lw * C * 128, [[128, 128], [128 * 128, NCT], [1, 128]]))
                wv_sb = wqkv_pool.tile([128, NCT, 128], BF, tag="wv", name="wv_sb")
                nc.sync.dma_start(
                    out=wv_sb[:],
                    in_=_ap(wv_d, lw * C * 128, [[128, 128], [128 * 128, NCT], [1, 128]]))
                qb_t = misc.tile([128, 1], F32, tag="qb", name="qb_t")
                nc.sync.dma_start(out=qb_t[:], in_=_ap(qb_d, lw * 128, [[1, 128], [1, 1]]))
                kb_t = misc.tile([128, 1], F32, tag="kb", name="kb_t")
                nc.sync.dma_start(out=kb_t[:], in_=_ap(kb_d, lw * 128, [[1, 128], [1, 1]]))

                hT_tiles = emit_ln()
                hfull = gather_hT(hT_tiles, f"hfull{l}")

                # ---- QKV for own heads over ALL tokens ----
                qT_sb = qkpool.tile([128, NCORES, TL], BF, tag="qT", name="qT_sb")
                kT_sb = qkpool.tile([128, NCORES, TL], BF, tag="kT", name="kT_sb")
                for r in range(NCORES):
                    ch = big8.tile([128, NCT, 512], BF, tag="big8", name="hch")
                    nc.sync.dma_start(
                        out=ch[:],
                        in_=_ap(hfull, r * HT_SZ, [[TL, 128], [128 * TL, NCT], [1, TL]]))
                    psK = ps_acc.tile([128, 512], F32, tag="acc", name="psK")
                    psQ = ps_acc.tile([128, 512], F32, tag="acc", name="psQ")
                    for ct in range(NCT):
                        nc.tensor.matmul(psK[:], wk_sb[:, ct, :], ch[:, ct, :],
                                         start=(ct == 0), stop=(ct == NCT - 1))
                        nc.tensor.matmul(psQ[:], wq_sb[:, ct, :], ch[:, ct, :],
                                         start=(ct == 0), stop=(ct == NCT - 1))
                    nc.scalar.activation(kT_sb[:, r, :], psK[:], AF.Identity, bias=kb_t[:])
                    nc.scalar.activation(qT_sb[:, r, :], psQ[:], AF.Identity, bias=qb_t[:])
                    # V groups must be sequential: a start=True zeroes the whole
                    # 2KB PSUM bank, so slices of one bank cannot accumulate
                    # concurrently
                    for s in range(SEQ):
                        psV = ps_sm.tile([128, SEQ, 128], F32, tag="sm", name="psV")
                        for ct in range(NCT):
                            nc.tensor.matmul(
                                psV[:, 0, :], ch[:, ct, s * 128:(s + 1) * 128], wv_sb[:, ct, :],
                                start=(ct == 0), stop=(ct == NCT - 1))
                        nc.vector.tensor_copy(
                            out=v_sb[:, r, s, :, 0:HD],
                            in_=psV[:, 0, :].rearrange("p (h d) -> p h d", h=2))

                if debug and l == 0:
                    nc.gpsimd.dma_start(out=dbgq_d[:], in_=qT_sb[:])
                    nc.gpsimd.dma_start(out=dbgk_d[:], in_=kT_sb[:])
                    nc.gpsimd.dma_start(out=dbgv_d[:], in_=v_sb[:])

                # ---- causal attention, 8 (s, hs) pairs ----
                oT_send = otsend.tile([128, NCORES, SEQ, 128], BF, tag="ots", name="oT_send")
                for s in range(SEQ):
                    oraw = orawp.tile([HD + 1, 2, NCORES, 128], F32, tag="oraw", name="oraw")
                    for hs in range(2):
                        p0 = hs * HD
                        pT = ptpool.tile([128, NSLOT, 128], BF, tag="pt", name="pT")
                        for kt in range(NCORES):
                            segs = [(kt, 4), (4, 8)] if kt < 4 else [(kt, 8)]
                            for (a, b) in segs:
                                w = (b - a) * 128
                                st = ps_acc.tile([128, 512], F32, tag="acc", name="st")
                                nc.tensor.matmul(
                                    st[:, 0:w],
                                    kT_sb[p0:p0 + HD, kt, s * 128:(s + 1) * 128],
                                    qT_sb[p0:p0 + HD, a:b, s * 128:(s + 1) * 128],
                                    start=True, stop=True)
                                sl = _SLOT[(kt, a)]
                                nc.scalar.activation(
                                    pT[:, sl:sl + b - a, :], st[:, 0:w], AF.Exp)
                            sl = _SLOT[(kt, kt)]
                            nc.vector.tensor_mul(
                                out=pT[:, sl, :], in0=pT[:, sl, :], in1=maskD[:])
                        for qt in range(NCORES):
                            ovt = ps_sm.tile([128, SEQ, 128], F32, tag="sm", name="ov")
                            ov = ovt[:, 0, :]
                            for kt in range(qt + 1):
                                sl = _SLOT[(kt, qt)]
                                nc.tensor.matmul(
                                    ovt[0:HD + 1, 0, :],
                                    v_sb[:, kt, s, hs, :],
                                    pT[:, sl, :],
                                    start=(kt == 0), stop=(kt == qt))
                            nc.vector.tensor_copy(out=oraw[:, hs, qt, :], in_=ovt[0:HD + 1, 0, :])
                    # denominators -> reciprocal in place -> bounce-broadcast
                    nc.vector.reciprocal(oraw[HD:HD + 1, :, :, :], oraw[HD:HD + 1, :, :, :])
                    nc.sync.dma_start(out=rc_bounce[s, :], in_=oraw[HD:HD + 1, :, :, :])
                    rb = rbp.tile([HD, 2, NCORES, 128], F32, tag="rb", name="rb")
                    nc.gpsimd.dma_start(
                        out=rb[:], in_=_ap(rc_bounce, s * 2 * NCORES * 128,
                                           [[0, HD], [1, 2 * NCORES * 128]]))
                    oS = osp.tile([HD, NCORES, 128], BF, tag="oS", name="oS")
                    for hs in range(2):
                        for qt in range(NCORES):
                            dst = oT_send[0:HD, qt, s, :] if hs == 0 else oS[:, qt, :]
                            nc.vector.tensor_mul(
                                out=dst, in0=oraw[0:HD, hs, qt, :], in1=rb[:, hs, qt, :])
                    nc.sync.dma_start(out=oT_send[HD:128, :, s, :], in_=oS[:])

                # ---- O exchange: AllToAll, blocks pre-chunked by consumer ----
                for qt in range(NCORES):
                    nc.sync.dma_start(
                        out=_ap(o_loc, qt * 128 * TL, [[TL, 128], [1, TL]]),
                        in_=oT_send[:, qt, :, :])
                o_x = dram.tile([O_SZ], BF, addr_space="Local", name=f"ox{l}")
                if sim:
                    nc.sync.dma_start(
                        out=_ap(o_x, 0, [[2048, O_SZ // 2048], [1, 2048]]),
                        in_=_ap(o_loc, 0, [[2048, O_SZ // 2048], [1, 2048]]))
                else:
                    nc.gpsimd.collective_compute(
                        "AllToAll",
                        mybir.AluOpType.bypass,
                        replica_groups=[list(range(NCORES))],
                        ins=[_ap(o_loc, 0, [[2048, O_SZ // 2048], [1, 2048]])],
                        outs=[_ap(o_x, 0, [[2048, O_SZ // 2048], [1, 2048]])])

                # assemble oT [128 p, ct, 512 t']: block r rows 0-63 = head r,
                # rows 64-127 = head r+8
                oT = otpool.tile([128, NCT, TL], BF, tag="oT", name="oT")
                for r in range(NCORES):
                    for g in range(2):
                        nc.sync.dma_start(
                            out=oT[(r % 2) * HD:(r % 2) * HD + HD, r // 2 + 4 * g, :],
                            in_=_ap(o_x, r * 128 * TL + g * HD * TL, [[TL, HD], [1, TL]]))

                if debug and l == 0:
                    nc.gpsimd.dma_start(out=dbgos_d[:], in_=oT_send[:])
                    nc.gpsimd.dma_start(out=dbgot_d[:], in_=oT[:])

                # ---- Wo + residual ----
                bo_bc = gbpool.tile([128, C], F32, tag="bo", name="bo_bc")
                bcast_row(bo_bc, bo_d, lw * C, C)
                for nf in range(2):
                    wos = big8.tile([128, NCT, 512], BF, tag="big8", name="wos")
                    nc.sync.dma_start(
                        out=wos[:],
                        in_=_ap(wo_d, lw * C * C + nf * 512,
                                [[C, 128], [128 * C, NCT], [1, 512]]))
                    pss = [ps_acc.tile([128, 512], F32, tag="acc", name="po") for _ in range(NT)]
                    for ct in range(NCT):
                        for tt in range(NT):
                            nc.tensor.matmul(
                                pss[tt][:], oT[:, ct, tt * 128:(tt + 1) * 128], wos[:, ct, :],
                                start=(ct == 0), stop=(ct == NCT - 1))
                    for tt in range(NT):
                        xs = x_t[tt][:, nf * 512:(nf + 1) * 512]
                        nc.vector.tensor_add(out=xs, in0=xs, in1=pss[tt][:])
                        nc.vector.tensor_add(out=xs, in0=xs, in1=bo_bc[:, nf * 512:(nf + 1) * 512])

                # ---- FFN ----
                b1_t = misc.tile([128, FF // 128], F32, tag="b1", name="b1_t")
                nc.gpsimd.dma_start(
                    out=b1_t[:], in_=_ap(b1_d, lw * FF, [[1, 128], [128, FF // 128]]))
                h2T = emit_ln()
                ug = ugpool.tile([128, FF // 128, 512], BF, tag="ug", name="ug")
                for fg in range(8):
                    w1s = big8.tile([128, NCT, 512], BF, tag="big8", name="w1s")
                    nc.sync.dma_start(
                        out=w1s[:],
                        in_=_ap(w1_d, lw * C * FF + fg * 512,
                                [[FF, 128], [128 * FF, NCT], [1, 512]]))
                    pss = [ps_acc.tile([128, 512], F32, tag="acc", name="pf") for _ in range(4)]
                    for ct in range(NCT):
                        for f4 in range(4):
                            nc.tensor.matmul(
                                pss[f4][:], w1s[:, ct, f4 * 128:(f4 + 1) * 128], h2T[ct][:],
                                start=(ct == 0), stop=(ct == NCT - 1))
                    for f4 in range(4):
                        ft = fg * 4 + f4
                        nc.scalar.activation(
                            ug[:, ft, :], pss[f4][:], AF.Gelu, bias=b1_t[:, ft:ft + 1])

                b2_bc = gbpool.tile([128, C], F32, tag="b2", name="b2_bc")
                bcast_row(b2_bc, b2_d, lw * C, C)
                for nf in range(2):
                    pss = [ps_acc.tile([128, 512], F32, tag="acc", name="p2") for _ in range(NT)]
                    for ftg in range(4):
                        w2s = big8.tile([128, 8, 512], BF, tag="big8", name="w2s")
                        nc.sync.dma_start(
                            out=w2s[:],
                            in_=_ap(w2_d, lw * FF * C + ftg * 8 * 128 * C + nf * 512,
                                    [[C, 128], [128 * C, 8], [1, 512]]))
                        for f8 in range(8):
                            ft = ftg * 8 + f8
                            for tt in range(NT):
                                nc.tensor.matmul(
                                    pss[tt][:], ug[:, ft, tt * 128:(tt + 1) * 128], w2s[:, f8, :],
                                    start=(ft == 0), stop=(ft == FF // 128 - 1))
                    for tt in range(NT):
                        xs = x_t[tt][:, nf * 512:(nf + 1) * 512]
                        nc.vector.tensor_add(out=xs, in0=xs, in1=pss[tt][:])
                        nc.vector.tensor_add(out=xs, in0=xs, in1=b2_bc[:, nf * 512:(nf + 1) * 512])

                if debug:
                    for tt in range(NT):
                        nc.sync.dma_start(
                            out=dbg_d[l, tt * 128:(tt + 1) * 128, :], in_=x_t[tt][:])

            # ---- final LN, gather, lm_head ----
            hfT = emit_ln()
            hffull = gather_hT(hfT, "hffull")
            blm_sb = misc.tile([128, 32], F32, tag="blm", name="blm_sb")
            nc.sync.dma_start(out=blm_sb[:], in_=_ap(blm_d, 0, [[1, 128], [128, 32]]))
            for vg in range(4):
                wlm_sb = wlmpool.tile([128, NCT, 8, 128], BF, tag="wlm", name="wlm_sb")
                nc.sync.dma_start(
                    out=wlm_sb[:],
                    in_=_ap(wlm_d, vg * NCT * 128 * 1024,
                            [[1024, 128], [128 * 1024, NCT], [1, 1024]]))
                for tg in range(NCORES):
                    hfch = big8.tile([128, NCT, 512], BF, tag="big8", name="hfch")
                    nc.sync.dma_start(
                        out=hfch[:],
                        in_=_ap(hffull, tg * HT_SZ, [[TL, 128], [128 * TL, NCT], [1, TL]]))
                    for vt in range(8):
                        ps = ps_acc.tile([128, 512], F32, tag="acc", name="plm")
                        for ct in range(NCT):
                            nc.tensor.matmul(
                                ps[:], wlm_sb[:, ct, vt, :], hfch[:, ct, :],
                                start=(ct == 0), stop=(ct == NCT - 1))
                        lg = lgout.tile([128, 512], F32, tag="lg", name="lg")
                        nc.scalar.activation(
                            lg[:], ps[:], AF.Identity,
                            bias=blm_sb[:, vg * 8 + vt:vg * 8 + vt + 1])
                        row0 = (vg * 8 + vt) * 128
                        nc.sync.dma_start(
                            out=logits_d[row0:row0 + 128, tg * 512:(tg + 1) * 512],
                            in_=lg[:])

    nc.compile()
    _prog_cache[key] = nc
    return nc


def _prep_inputs(inputs):
    f = {k: np.asarray(v) for k, v in inputs.items()}
    idx = f["idx"].astype(np.int64)
    emb = f["emb"].astype(np.float32)
    pos = f["pos_enc"].astype(np.float32)
    x_full = emb[idx] + pos[None, :, :]          # [B, T, C] f32

    scale = HD ** -0.5
    bf = lambda a: np.ascontiguousarray(a, dtype=np.float32).astype(BF16NP)
    f32 = lambda a: np.ascontiguousarray(a, dtype=np.float32)

    Wq, Wk, Wv, Wo = (f[k].astype(np.float32) for k in ("Wq", "Wk", "Wv", "Wo"))
    W1, W2 = f["W1"].astype(np.float32), f["W2"].astype(np.float32)
    g1, b1n = f["ln1_g"].astype(np.float32), f["ln1_b"].astype(np.float32)
    g2, b2n = f["ln2_g"].astype(np.float32), f["ln2_b"].astype(np.float32)
    gf, bfn = f["lnf_g"].astype(np.float32), f["lnf_b"].astype(np.float32)
    bo, b1, b2 = (f[k].astype(np.float32) for k in ("bo", "b1", "b2"))
    Wlm = f["Wlm"].astype(np.float32)
    blm = f["blm"].astype(np.float32)

    # fold LN gains into weight rows, LN biases into per-channel output biases
    wqf = g1[:, :, None] * Wq * scale            # [L, C, C]
    wkf = g1[:, :, None] * Wk
    wvf = g1[:, :, None] * Wv
    qb_full = np.einsum("lc,lcd->ld", b1n, Wq) * scale   # [L, C]
    kb_full = np.einsum("lc,lcd->ld", b1n, Wk)
    vb_full = np.einsum("lc,lcd->ld", b1n, Wv)
    bo_eff = bo + np.einsum("lc,lcd->ld", vb_full, Wo)   # V bias folded thru Wo
    w1f = g2[:, :, None] * W1
    b1_eff = b1 + np.einsum("lc,lcf->lf", b2n, W1)
    wlmf = gf[:, None] * Wlm                     # [C, V]
    blm_eff = blm + bfn @ Wlm                    # [V]

    # causal mask for diagonal tiles, S^T layout [k, q]
    kk = np.arange(128)[:, None]
    jj = np.arange(128)[None, :]
    maskD = (kk <= jj).astype(np.float32).astype(BF16NP)

    in_maps = []
    for c in range(NCORES):
        cols = np.r_[c * HD:(c + 1) * HD, (c + 8) * HD:(c + 9) * HD]
        x0_c = np.ascontiguousarray(
            x_full[:, 128 * c:128 * (c + 1), :].reshape(TL, C), dtype=np.float32)

        # lm shard, folded + padded to 4096, tiled [vg][ct][cp][vt*128]
        wlm_c = wlmf[:, c * VSH:(c + 1) * VSH]
        wlm_pad = np.zeros((C, VPAD), np.float32)
        wlm_pad[:, :VSH] = wlm_c
        # [vg 4][ct 8][cp 128][vt 8 *128]
        wlm_t = wlm_pad.reshape(C, 4, 1024).transpose(1, 0, 2)     # [vg, C, 1024]
        wlm_t = wlm_t.reshape(4, NCT, 128, 1024)
        blm_pad = np.zeros((VPAD,), np.float32)
        blm_pad[:VSH] = blm_eff[c * VSH:(c + 1) * VSH]

        im = {
            "x0": x0_c,
            "maskD": maskD,
            "wq": bf(wqf[:, :, cols]),
            "wk": bf(wkf[:, :, cols]),
            "wv": bf(wvf[:, :, cols]),
            "qb": f32(qb_full[:, cols]),
            "kb": f32(kb_full[:, cols]),
            "wo": bf(Wo),
            "bo": f32(bo_eff),
            "w1": bf(w1f),
            "b1": f32(b1_eff),
            "w2": bf(W2),
            "b2": f32(b2),
            "wlm": np.ascontiguousarray(wlm_t).astype(BF16NP),
            "blm": f32(blm_pad),
        }
        in_maps.append(im)
    return in_maps


def kernel(**inputs):
    nc = _build()
    in_maps = _prep_inputs(inputs)
    res = run_bass_kernel_spmd(nc, in_maps, list(range(NCORES)))
    # per-core logits^T [VPAD, 4096 t]; t = (r, s, j); vocab sharded on cores
    parts = []
    for r in res.results:
        lg = r["logits"][:VSH]                      # [4000, 4096]
        lg = lg.reshape(VSH, NCORES, SEQ, 128).transpose(2, 1, 3, 0)
        parts.append(lg.reshape(SEQ, T, VSH))       # [s, t, 4000]
    full = np.concatenate(parts, axis=-1)           # [B, T, V]
    return np.ascontiguousarray(full, dtype=np.float32)


# revision 12
# speedup vs baseline: 1.1594x; 1.0280x over previous
"""GPT forward pass on 8 Trainium2 NeuronCores — v2.

Sharding:
  - Residual/trunk GEMMs (Wo, FFN) are token-parallel: core c owns q-tile c
    (rows 128c..128c+127) of each of the 4 sequences (512 tokens/core).
  - QKV projection + attention are head-sharded: core c owns heads {c, c+8}
    over ALL 4096 tokens. Per layer the post-LN hidden states h^T are
    AllGathered (1 MB/rank, bf16); each core computes Q/K/V for its two heads
    over all tokens and runs full causal attention for its 8 (seq, head)
    pairs, skipping above-diagonal 128x128 tiles. Attention outputs return to
    token owners via AllToAll (1 MB/rank), pre-chunked by consumer.
  - lm_head is vocab-sharded (4000 cols/core, padded to 4096).

LayerNorm gains/biases are folded into weights host-side (g into W rows, b
into per-output-channel biases), so on-chip LN is just (x-mean)*rstd. The
V-path bias is folded into bo via bo' = bo + (b@Wv)@Wo. Softmax denominators
come free from a ones-column appended to V; exp runs on ScalarE over causal
tiles only. All matmuls bf16 with fp32 PSUM; residual stream fp32.
"""

import os
import sys

for _p in ("/opt/trn_rl_repo",):
    if os.path.isdir(_p) and _p not in sys.path:
        sys.path.insert(0, _p)

import numpy as np
import ml_dtypes

BF16NP = ml_dtypes.bfloat16

import concourse.bass as bass
import concourse.mybir as mybir
import concourse.tile as tile
from concourse import bacc
from concourse.bass_utils import run_bass_kernel_spmd
from concourse.masks import make_identity

F32 = mybir.dt.float32
BF = mybir.dt.bfloat16
AF = mybir.ActivationFunctionType

V, C, T, H, L, B = 32000, 1024, 1024, 16, 4, 4
HD = C // H          # 64
FF = 4 * C           # 4096
NCORES = 8
TL = 512             # local tokens per core (4 seqs x 128)
SEQ = B              # 4
NT = TL // 128       # 4
NCT = C // 128       # 8
VSH = V // NCORES    # 4000
VPAD = 4096          # padded vocab shard
LN_EPS = 1e-5

HT_SZ = C * TL            # one rank's h^T payload elems (512K)
O_SZ = NCORES * 128 * TL  # o payload elems per rank (8 consumer chunks)

# packed causal slot table: slot(kt, qt) defined for kt <= qt
_SLOT = {}
_off = 0
for _kt in range(8):
    for _qt in range(_kt, 8):
        _SLOT[(_kt, _qt)] = _off
        _off += 1
NSLOT = _off  # 36

_prog_cache = {}


def _ap(t, offset, pattern):
    return bass.AP(tensor=t.tensor if isinstance(t, bass.AP) else t, offset=offset, ap=pattern)


def _build(LL=L, debug=False, sim=False):
    key = (LL, debug, sim)
    if key in _prog_cache:
        return _prog_cache[key]

    nc = bacc.Bacc("TRN2", target_bir_lowering=False, debug=False, num_devices=NCORES)

    x0_d = nc.dram_tensor("x0", [TL, C], F32, kind="ExternalInput")
    maskD_d = nc.dram_tensor("maskD", [128, 128], BF, kind="ExternalInput")
    wq_d = nc.dram_tensor("wq", [L, C, 128], BF, kind="ExternalInput")
    wk_d = nc.dram_tensor("wk", [L, C, 128], BF, kind="ExternalInput")
    wv_d = nc.dram_tensor("wv", [L, C, 128], BF, kind="ExternalInput")
    qb_d = nc.dram_tensor("qb", [L, 128], F32, kind="ExternalInput")
    kb_d = nc.dram_tensor("kb", [L, 128], F32, kind="ExternalInput")
    wo_d = nc.dram_tensor("wo", [L, C, C], BF, kind="ExternalInput")
    bo_d = nc.dram_tensor("bo", [L, C], F32, kind="ExternalInput")
    w1_d = nc.dram_tensor("w1", [L, C, FF], BF, kind="ExternalInput")
    b1_d = nc.dram_tensor("b1", [L, FF], F32, kind="ExternalInput")
    w2_d = nc.dram_tensor("w2", [L, FF, C], BF, kind="ExternalInput")
    b2_d = nc.dram_tensor("b2", [L, C], F32, kind="ExternalInput")
    # wlm layout: [vg 4][ct 8][cp 128][vt 8 * 128 v]
    wlm_d = nc.dram_tensor("wlm", [4, NCT, 128, 1024], BF, kind="ExternalInput")
    blm_d = nc.dram_tensor("blm", [VPAD], F32, kind="ExternalInput")

    logits_d = nc.dram_tensor("logits", [VPAD, NCORES * TL], F32, kind="ExternalOutput")
    dbg_d = None
    if debug:
        dbg_d = nc.dram_tensor("dbg", [LL, TL, C], F32, kind="ExternalOutput")
        dbgq_d = nc.dram_tensor("dbgq", [128, NCORES, TL], F32, kind="ExternalOutput")
        dbgk_d = nc.dram_tensor("dbgk", [128, NCORES, TL], F32, kind="ExternalOutput")
        dbgv_d = nc.dram_tensor("dbgv", [128, NCORES, SEQ, 2, HD + 1], F32, kind="ExternalOutput")
        dbgos_d = nc.dram_tensor("dbgos", [128, NCORES, SEQ, 128], F32, kind="ExternalOutput")
        dbgot_d = nc.dram_tensor("dbgot", [128, NCT, TL], F32, kind="ExternalOutput")

    with tile.TileContext(nc) as tc:
        import contextlib

        with contextlib.ExitStack() as ctx:
            # SBUF pools (per-partition KB in comments)
            const = ctx.enter_context(tc.tile_pool(name="const", bufs=1))      # 0.7
            xpool = ctx.enter_context(tc.tile_pool(name="x", bufs=1))          # 16
            hpool = ctx.enter_context(tc.tile_pool(name="h", bufs=1))          # 8
            tpool = ctx.enter_context(tc.tile_pool(name="hT", bufs=1))         # 8
            wqkv_pool = ctx.enter_context(tc.tile_pool(name="wqkv", bufs=1))   # 6
            big8 = ctx.enter_context(tc.tile_pool(name="big8", bufs=3))        # 24
            qkpool = ctx.enter_context(tc.tile_pool(name="qk", bufs=1))        # 16
            vpool = ctx.enter_context(tc.tile_pool(name="v", bufs=1))          # 8.3
            ptpool = ctx.enter_context(tc.tile_pool(name="pt", bufs=2))        # 18
            orawp = ctx.enter_context(tc.tile_pool(name="oraw", bufs=1))       # 8
            rbp = ctx.enter_context(tc.tile_pool(name="rb", bufs=1))           # 8
            osp = ctx.enter_context(tc.tile_pool(name="oS", bufs=1))           # 2
            otsend = ctx.enter_context(tc.tile_pool(name="otsend", bufs=1))    # 8
            otpool = ctx.enter_context(tc.tile_pool(name="oT", bufs=1))        # 8
            ugpool = ctx.enter_context(tc.tile_pool(name="ug", bufs=1))        # 32
            wlmpool = ctx.enter_context(tc.tile_pool(name="wlm", bufs=2))      # 32
            lgout = ctx.enter_context(tc.tile_pool(name="lgout", bufs=2))      # 4
            gbpool = ctx.enter_context(tc.tile_pool(name="gb", bufs=1))        # 8
            misc = ctx.enter_context(tc.tile_pool(name="misc", bufs=2))        # 1.5
            # PSUM: acc 4x[128,512]f32 (4 banks) + sm 8x[128,128]f32 (2 banks)
            # + tp 2x[128,512]bf16 (1 bank) = 7 of 8 banks
            ps_acc = ctx.enter_context(tc.tile_pool(name="psacc", bufs=4, space="PSUM"))
            ps_sm = ctx.enter_context(tc.tile_pool(name="pssm", bufs=2, space="PSUM"))
            ps_tp = ctx.enter_context(tc.tile_pool(name="pstp", bufs=2, space="PSUM"))
            dram = ctx.enter_context(tc.tile_pool(name="dram", bufs=1, space="DRAM"))

            ident = const.tile([128, 128], BF, name="ident")
            make_identity(nc, ident)
            eps_t = const.tile([128, 1], F32, name="eps")
            nc.vector.memset(eps_t[:], LN_EPS)
            maskD = const.tile([128, 128], BF, name="maskD")
            nc.sync.dma_start(out=maskD[:], in_=maskD_d[:])

            hT_loc = dram.tile([HT_SZ], BF, name="hT_loc")
            o_loc = dram.tile([O_SZ], BF, name="o_loc")
            rc_bounce = dram.tile([SEQ, 2 * NCT * 128], F32, name="rc_bounce")

            x_t = [xpool.tile([128, C], F32, tag=f"x{tt}", name=f"x{tt}") for tt in range(NT)]
            for tt in range(NT):
                nc.sync.dma_start(out=x_t[tt][:], in_=x0_d[tt * 128:(tt + 1) * 128, :])

            def bcast_row(dst, src_1d, offset, n):
                src = _ap(src_1d, offset, [[0, dst.shape[0]], [1, n]])
                nc.gpsimd.dma_start(out=dst[:], in_=src)

            def emit_ln():
                """(x - mean) * rstd -> bf16, transposed into 8 hT tiles."""
                h_tiles = []
                for tt in range(NT):
                    stats = misc.tile([128, 2, 6], F32, name="stats", tag="stats")
                    xv = x_t[tt][:].rearrange("p (s d) -> p s d", s=2)
                    nc.vector.bn_stats(out=stats[:, 0, :], in_=xv[:, 0, :])
                    nc.vector.bn_stats(out=stats[:, 1, :], in_=xv[:, 1, :])
                    mv = misc.tile([128, 2], F32, name="mv", tag="mv")
                    nc.vector.bn_aggr(out=mv[:], in_=stats[:])
                    rstd = misc.tile([128, 1], F32, name="rstd", tag="rstd")
                    nc.scalar.activation(rstd[:], mv[:, 1:2], AF.Sqrt, bias=eps_t[:])
                    nc.vector.reciprocal(rstd[:], rstd[:])
                    h = hpool.tile([128, C], BF, tag=f"h{tt}", name=f"h{tt}")
                    nc.vector.tensor_scalar(
                        out=h[:], in0=x_t[tt][:], scalar1=mv[:, 0:1], scalar2=rstd[:],
                        op0=mybir.AluOpType.subtract, op1=mybir.AluOpType.mult,
                    )
                    h_tiles.append(h)
                hT = tpool.tile([128, NCT, 512], BF, tag="hT", name="hT")
                for ct in range(NCT):
                    pst = ps_tp.tile([128, 512], BF, tag="tp", name="pst")
                    for tt in range(NT):
                        nc.tensor.transpose(
                            pst[:, tt * 128:(tt + 1) * 128],
                            h_tiles[tt][:, ct * 128:(ct + 1) * 128],
                            ident[:],
                        )
                    nc.vector.tensor_copy(out=hT[:, ct, :], in_=pst[:])
                return hT

            def gather_hT(hT_tiles, lname):
                """write local h^T -> DRAM, AllGather -> [r][ct][cp][512]."""
                nc.sync.dma_start(
                    out=_ap(hT_loc, 0, [[TL, 128], [128 * TL, NCT], [1, TL]]),
                    in_=hT_tiles[:],
                )
                full = dram.tile([NCORES * HT_SZ], BF,
                                 addr_space="Local" if sim else "Shared", name=lname)
                if sim:
                    nc.sync.dma_start(
                        out=_ap(full, 0, [[2048, HT_SZ // 2048], [1, 2048]]),
                        in_=_ap(hT_loc, 0, [[2048, HT_SZ // 2048], [1, 2048]]),
                    )
                else:
                    nc.gpsimd.collective_compute(
                        "AllGather",
                        mybir.AluOpType.bypass,
                        replica_groups=[list(range(NCORES))],
                        ins=[_ap(hT_loc, 0, [[2048, HT_SZ // 2048], [1, 2048]])],
                        outs=[_ap(full, 0, [[2048, NCORES * HT_SZ // 2048], [1, 2048]])],
                    )
                return full

            # v_sb: [128 j, kt 8, s 4, hs 2, 65]; col 64 is the ones column
            v_sb = vpool.tile([128, NCORES, SEQ, 2, HD + 1], BF, name="v_sb")
            nc.vector.memset(v_sb[:, :, :, :, HD:HD + 1], 1.0)

            for l in range(LL):
                lw = l % L
                wq_sb = wqkv_pool.tile([128, NCT, 128], BF, tag="wq", name="wq_sb")
                nc.sync.dma_start(
                    out=wq_sb[:],
                    in_=_ap(wq_d, lw * C * 128, [[128, 128], [128 * 128, NCT], [1, 128]]))
                wk_sb = wqkv_pool.tile([128, NCT, 128], BF, tag="wk", name="wk_sb")
                nc.sync.dma_start(
                    out=wk_sb[:],
                    in_=_ap(wk_d, lw * C * 128, [[128, 128], [128 * 128, NCT], [1, 128]]))
                wv_sb = wqkv_pool.tile([128, NCT, 128], BF, tag="wv", name="wv_sb")
                nc.sync.dma_start(
                    out=wv_sb[:],
                    in_=_ap(wv_d, lw * C * 128, [[128, 128], [128 * 128, NCT], [1, 128]]))
                qb_t = misc.tile([128, 1], F32, tag="qb", name="qb_t")
                nc.sync.dma_start(out=qb_t[:], in_=_ap(qb_d, lw * 128, [[1, 128], [1, 1]]))
                kb_t = misc.tile([128, 1], F32, tag="kb", name="kb_t")
                nc.sync.dma_start(out=kb_t[:], in_=_ap(kb_d, lw * 128, [[1, 128], [1, 1]]))

                hT_tiles = emit_ln()
                hfull = gather_hT(hT_tiles, f"hfull{l}")

                # ---- QKV for own heads over ALL tokens ----
                qT_sb = qkpool.tile([128, NCORES, TL], BF, tag="qT", name="qT_sb")
                kT_sb = qkpool.tile([128, NCORES, TL], BF, tag="kT", name="kT_sb")
                for r in range(NCORES):
                    ch = big8.tile([128, NCT, 512], BF, tag="big8", name="hch")
                    nc.sync.dma_start(
                        out=ch[:],
                        in_=_ap(hfull, r * HT_SZ, [[TL, 128], [128 * TL, NCT], [1, TL]]))
                    psK = ps_acc.tile([128, 512], F32, tag="acc", name="psK")
                    psQ = ps_acc.tile([128, 512], F32, tag="acc", name="psQ")
                    for ct in range(NCT):
                        nc.tensor.matmul(psK[:], wk_sb[:, ct, :], ch[:, ct, :],
                                         start=(ct == 0), stop=(ct == NCT - 1))
                        nc.tensor.matmul(psQ[:], wq_sb[:, ct, :], ch[:, ct, :],
                                         start=(ct == 0), stop=(ct == NCT - 1))
                    nc.scalar.activation(kT_sb[:, r, :], psK[:], AF.Identity, bias=kb_t[:])
                    nc.scalar.activation(qT_sb[:, r, :], psQ[:], AF.Identity, bias=qb_t[:])
                    # V groups must be sequential: a start=True zeroes the whole
                    # 2KB PSUM bank, so slices of one bank cannot accumulate
                    # concurrently
                    for s in range(SEQ):
                        psV = ps_sm.tile([128, SEQ, 128], F32, tag="sm", name="psV")
                        for ct in range(NCT):
                            nc.tensor.matmul(
                                psV[:, 0, :], ch[:, ct, s * 128:(s + 1) * 128], wv_sb[:, ct, :],
                                start=(ct == 0), stop=(ct == NCT - 1))
                        nc.vector.tensor_copy(
                            out=v_sb[:, r, s, :, 0:HD],
                            in_=psV[:, 0, :].rearrange("p (h d) -> p h d", h=2))

                if debug and l == 0:
                    nc.gpsimd.dma_start(out=dbgq_d[:], in_=qT_sb[:])
                    nc.gpsimd.dma_start(out=dbgk_d[:], in_=kT_sb[:])
                    nc.gpsimd.dma_start(out=dbgv_d[:], in_=v_sb[:])

                # ---- causal attention, 8 (s, hs) pairs ----
                oT_send = otsend.tile([128, NCORES, SEQ, 128], BF, tag="ots", name="oT_send")
                for s in range(SEQ):
                    oraw = orawp.tile([HD + 1, 2, NCORES, 128], F32, tag="oraw", name="oraw")
                    pTs = {}
                    for hs in range(2):
                        p0 = hs * HD
                        pT = ptpool.tile([128, NSLOT, 128], BF, tag="pt", name="pT")
                        pTs[hs] = pT
                        for kt in range(NCORES):
                            segs = [(kt, 4), (4, 8)] if kt < 4 else [(kt, 8)]
                            for (a, b) in segs:
                                w = (b - a) * 128
                                diag = a == kt
                                st = ps_acc.tile([128, 512], F32, tag="acc", name="st")
                                nc.tensor.matmul(
                                    st[:, 0:w],
                                    kT_sb[p0:p0 + HD, kt, s * 128:(s + 1) * 128],
                                    qT_sb[p0:p0 + HD, a:b, s * 128:(s + 1) * 128],
                                    start=True, stop=not diag)
                                if diag:
                                    # add -1e4 above the diagonal (exp -> 0)
                                    nc.tensor.matmul(
                                        st[:, 0:128], maskD[:], ident[:],
                                        start=False, stop=True)
                                sl = _SLOT[(kt, a)]
                                nc.scalar.activation(
                                    pT[:, sl:sl + b - a, :], st[:, 0:w], AF.Exp)
                    for hs in range(2):
                        pT = pTs[hs]
                        for qt in range(NCORES):
                            ovt = ps_sm.tile([128, SEQ, 128], F32, tag="sm", name="ov")
                            for kt in range(qt + 1):
                                sl = _SLOT[(kt, qt)]
                                nc.tensor.matmul(
                                    ovt[0:HD + 1, 0, :],
                                    v_sb[:, kt, s, hs, :],
                                    pT[:, sl, :],
                                    start=(kt == 0), stop=(kt == qt))
                            nc.vector.tensor_copy(out=oraw[:, hs, qt, :], in_=ovt[0:HD + 1, 0, :])
                    # denominators -> reciprocal in place -> bounce-broadcast
                    nc.vector.reciprocal(oraw[HD:HD + 1, :, :, :], oraw[HD:HD + 1, :, :, :])
                    nc.sync.dma_start(out=rc_bounce[s, :], in_=oraw[HD:HD + 1, :, :, :])
                    rb = rbp.tile([HD, 2, NCORES, 128], BF, tag="rb", name="rb")
                    nc.gpsimd.dma_start(
                        out=rb[:], in_=_ap(rc_bounce, s * 2 * NCORES * 128,
                                           [[0, HD], [1, 2 * NCORES * 128]]))
                    oS = osp.tile([HD, NCORES, 128], BF, tag="oS", name="oS")
                    for hs in range(2):
                        for qt in range(NCORES):
                            dst = oT_send[0:HD, qt, s, :] if hs == 0 else oS[:, qt, :]
                            nc.vector.tensor_mul(
                                out=dst, in0=oraw[0:HD, hs, qt, :], in1=rb[:, hs, qt, :])
                    nc.sync.dma_start(out=oT_send[HD:128, :, s, :], in_=oS[:])

                # prefetch Wo weight halves while the exchange is in flight
                wos_t = []
                for nf in range(2):
                    wos = big8.tile([128, NCT, 512], BF, tag="big8", name="wos")
                    nc.sync.dma_start(
                        out=wos[:],
                        in_=_ap(wo_d, lw * C * C + nf * 512,
                                [[C, 128], [128 * C, NCT], [1, 512]]))
                    wos_t.append(wos)

                # ---- O exchange: AllToAll, blocks pre-chunked by consumer ----
                nc.sync.dma_start(
                    out=_ap(o_loc, 0, [[TL, 128], [128 * TL, NCORES], [1, TL]]),
                    in_=oT_send[:].rearrange("p q s j -> p q (s j)"),
                )
                o_x = dram.tile([O_SZ], BF, addr_space="Local", name=f"ox{l}")
                if sim:
                    nc.sync.dma_start(
                        out=_ap(o_x, 0, [[2048, O_SZ // 2048], [1, 2048]]),
                        in_=_ap(o_loc, 0, [[2048, O_SZ // 2048], [1, 2048]]))
                else:
                    nc.gpsimd.collective_compute(
                        "AllToAll",
                        mybir.AluOpType.bypass,
                        replica_groups=[list(range(NCORES))],
                        ins=[_ap(o_loc, 0, [[2048, O_SZ // 2048], [1, 2048]])],
                        outs=[_ap(o_x, 0, [[2048, O_SZ // 2048], [1, 2048]])])

                # assemble oT [128 p, cc 4, g 2, 512 t'] (channel ct = g*4+cc):
                # block r rows 0-63 = head r (g=0), rows 64-127 = head r+8 (g=1)
                oT = otpool.tile([128, 4, 2, TL], BF, tag="oT", name="oT")
                for r in range(NCORES):
                    nc.sync.dma_start(
                        out=oT[(r % 2) * HD:(r % 2) * HD + HD, r // 2, :, :],
                        in_=_ap(o_x, r * 128 * TL, [[TL, HD], [HD * TL, 2], [1, TL]]))

                if debug and l == 0:
                    nc.gpsimd.dma_start(out=dbgos_d[:], in_=oT_send[:])
                    nc.gpsimd.dma_start(out=dbgot_d[:], in_=oT[:])

                # ---- Wo + residual ----
                bo_bc = gbpool.tile([128, C], BF, tag="bo", name="bo_bc")
                bcast_row(bo_bc, bo_d, lw * C, C)
                for nf in range(2):
                    wos = wos_t[nf]
                    pss = [ps_acc.tile([128, 512], F32, tag="acc", name="po") for _ in range(NT)]
                    for ct in range(NCT):
                        for tt in range(NT):
                            nc.tensor.matmul(
                                pss[tt][:], oT[:, ct % 4, ct // 4, tt * 128:(tt + 1) * 128],
                                wos[:, ct, :],
                                start=(ct == 0), stop=(ct == NCT - 1))
                    for tt in range(NT):
                        xs = x_t[tt][:, nf * 512:(nf + 1) * 512]
                        nc.vector.tensor_add(out=xs, in0=xs, in1=pss[tt][:])
                        nc.vector.tensor_add(out=xs, in0=xs, in1=bo_bc[:, nf * 512:(nf + 1) * 512])

                # ---- FFN ----
                b1_t = misc.tile([128, FF // 128], F32, tag="b1", name="b1_t")
                nc.gpsimd.dma_start(
                    out=b1_t[:], in_=_ap(b1_d, lw * FF, [[1, 128], [128, FF // 128]]))
                h2T = emit_ln()
                ug = ugpool.tile([128, FF // 128, 512], BF, tag="ug", name="ug")
                for fg in range(8):
                    w1s = big8.tile([128, NCT, 512], BF, tag="big8", name="w1s")
                    nc.sync.dma_start(
                        out=w1s[:],
                        in_=_ap(w1_d, lw * C * FF + fg * 512,
                                [[FF, 128], [128 * FF, NCT], [1, 512]]))
                    pss = [ps_acc.tile([128, 512], F32, tag="acc", name="pf") for _ in range(4)]
                    for ct in range(NCT):
                        for f4 in range(4):
                            nc.tensor.matmul(
                                pss[f4][:], w1s[:, ct, f4 * 128:(f4 + 1) * 128], h2T[:, ct, :],
                                start=(ct == 0), stop=(ct == NCT - 1))
                    for f4 in range(4):
                        ft = fg * 4 + f4
                        nc.scalar.activation(
                            ug[:, ft, :], pss[f4][:], AF.Gelu, bias=b1_t[:, ft:ft + 1])

                b2_bc = gbpool.tile([128, C], BF, tag="b2", name="b2_bc")
                bcast_row(b2_bc, b2_d, lw * C, C)
                for nf in range(2):
                    pss = [ps_acc.tile([128, 512], F32, tag="acc", name="p2") for _ in range(NT)]
                    for ftg in range(4):
                        w2s = big8.tile([128, 8, 512], BF, tag="big8", name="w2s")
                        nc.sync.dma_start(
                            out=w2s[:],
                            in_=_ap(w2_d, lw * FF * C + ftg * 8 * 128 * C + nf * 512,
                                    [[C, 128], [128 * C, 8], [1, 512]]))
                        for f8 in range(8):
                            ft = ftg * 8 + f8
                            for tt in range(NT):
                                nc.tensor.matmul(
                                    pss[tt][:], ug[:, ft, tt * 128:(tt + 1) * 128], w2s[:, f8, :],
                                    start=(ft == 0), stop=(ft == FF // 128 - 1))
                    for tt in range(NT):
                        xs = x_t[tt][:, nf * 512:(nf + 1) * 512]
                        nc.vector.tensor_add(out=xs, in0=xs, in1=pss[tt][:])
                        nc.vector.tensor_add(out=xs, in0=xs, in1=b2_bc[:, nf * 512:(nf + 1) * 512])

                if debug:
                    for tt in range(NT):
                        nc.sync.dma_start(
                            out=dbg_d[l, tt * 128:(tt + 1) * 128, :], in_=x_t[tt][:])

            # ---- final LN, gather, lm_head ----
            hfT = emit_ln()
            hffull = gather_hT(hfT, "hffull")
            blm_sb = misc.tile([128, 32], F32, tag="blm", name="blm_sb")
            nc.sync.dma_start(out=blm_sb[:], in_=_ap(blm_d, 0, [[1, 128], [128, 32]]))
            for vg in range(4):
                wlm_sb = wlmpool.tile([128, NCT, 8, 128], BF, tag="wlm", name="wlm_sb")
                nc.sync.dma_start(
                    out=wlm_sb[:],
                    in_=_ap(wlm_d, vg * NCT * 128 * 1024,
                            [[1024, 128], [128 * 1024, NCT], [1, 1024]]))
                for tg in range(NCORES):
                    hfch = big8.tile([128, NCT, 512], BF, tag="big8", name="hfch")
                    nc.sync.dma_start(
                        out=hfch[:],
                        in_=_ap(hffull, tg * HT_SZ, [[TL, 128], [128 * TL, NCT], [1, TL]]))
                    for vt in range(8):
                        ps = ps_acc.tile([128, 512], F32, tag="acc", name="plm")
                        for ct in range(NCT):
                            nc.tensor.matmul(
                                ps[:], wlm_sb[:, ct, vt, :], hfch[:, ct, :],
                                start=(ct == 0), stop=(ct == NCT - 1))
                        lg = lgout.tile([128, 512], F32, tag="lg", name="lg")
                        nc.scalar.activation(
                            lg[:], ps[:], AF.Identity,
                            bias=blm_sb[:, vg * 8 + vt:vg * 8 + vt + 1])
                        row0 = (vg * 8 + vt) * 128
                        nc.sync.dma_start(
                            out=logits_d[row0:row0 + 128, tg * 512:(tg + 1) * 512],
                            in_=lg[:])

    nc.compile()
    _prog_cache[key] = nc
    return nc


def _prep_inputs(inputs):
    f = {k: np.asarray(v) for k, v in inputs.items()}
    idx = f["idx"].astype(np.int64)
    emb = f["emb"].astype(np.float32)
    pos = f["pos_enc"].astype(np.float32)
    x_full = emb[idx] + pos[None, :, :]          # [B, T, C] f32

    scale = HD ** -0.5
    bf = lambda a: np.ascontiguousarray(a, dtype=np.float32).astype(BF16NP)
    f32 = lambda a: np.ascontiguousarray(a, dtype=np.float32)

    Wq, Wk, Wv, Wo = (f[k].astype(np.float32) for k in ("Wq", "Wk", "Wv", "Wo"))
    W1, W2 = f["W1"].astype(np.float32), f["W2"].astype(np.float32)
    g1, b1n = f["ln1_g"].astype(np.float32), f["ln1_b"].astype(np.float32)
    g2, b2n = f["ln2_g"].astype(np.float32), f["ln2_b"].astype(np.float32)
    gf, bfn = f["lnf_g"].astype(np.float32), f["lnf_b"].astype(np.float32)
    bo, b1, b2 = (f[k].astype(np.float32) for k in ("bo", "b1", "b2"))
    Wlm = f["Wlm"].astype(np.float32)
    blm = f["blm"].astype(np.float32)

    # fold LN gains into weight rows, LN biases into per-channel output biases
    wqf = g1[:, :, None] * Wq * scale            # [L, C, C]
    wkf = g1[:, :, None] * Wk
    wvf = g1[:, :, None] * Wv
    qb_full = np.einsum("lc,lcd->ld", b1n, Wq) * scale   # [L, C]
    kb_full = np.einsum("lc,lcd->ld", b1n, Wk)
    vb_full = np.einsum("lc,lcd->ld", b1n, Wv)
    bo_eff = bo + np.einsum("lc,lcd->ld", vb_full, Wo)   # V bias folded thru Wo
    w1f = g2[:, :, None] * W1
    b1_eff = b1 + np.einsum("lc,lcf->lf", b2n, W1)
    wlmf = gf[:, None] * Wlm                     # [C, V]
    blm_eff = blm + bfn @ Wlm                    # [V]

    # additive causal mask for diagonal tiles, supplied as lhsT = M^T where
    # M[k, q] = 0 if k <= q else -1e4 (exp -> 0); M^T[q, k] = M[k, q]
    kk = np.arange(128)[None, :]
    qq = np.arange(128)[:, None]
    maskD = np.where(kk <= qq, 0.0, -1e4).astype(np.float32).astype(BF16NP)

    in_maps = []
    for c in range(NCORES):
        cols = np.r_[c * HD:(c + 1) * HD, (c + 8) * HD:(c + 9) * HD]
        x0_c = np.ascontiguousarray(
            x_full[:, 128 * c:128 * (c + 1), :].reshape(TL, C), dtype=np.float32)

        # lm shard, folded + padded to 4096, tiled [vg][ct][cp][vt*128]
        wlm_c = wlmf[:, c * VSH:(c + 1) * VSH]
        wlm_pad = np.zeros((C, VPAD), np.float32)
        wlm_pad[:, :VSH] = wlm_c
        # [vg 4][ct 8][cp 128][vt 8 *128]
        wlm_t = wlm_pad.reshape(C, 4, 1024).transpose(1, 0, 2)     # [vg, C, 1024]
        wlm_t = wlm_t.reshape(4, NCT, 128, 1024)
        blm_pad = np.zeros((VPAD,), np.float32)
        blm_pad[:VSH] = blm_eff[c * VSH:(c + 1) * VSH]

        im = {
            "x0": x0_c,
            "maskD": maskD,
            "wq": bf(wqf[:, :, cols]),
            "wk": bf(wkf[:, :, cols]),
            "wv": bf(wvf[:, :, cols]),
            "qb": f32(qb_full[:, cols]),
            "kb": f32(kb_full[:, cols]),
            "wo": bf(Wo),
            "bo": f32(bo_eff),
            "w1": bf(w1f),
            "b1": f32(b1_eff),
            "w2": bf(W2),
            "b2": f32(b2),
            "wlm": np.ascontiguousarray(wlm_t).astype(BF16NP),
            "blm": f32(blm_pad),
        }
        in_maps.append(im)
    return in_maps


def kernel(**inputs):
    nc = _build()
    in_maps = _prep_inputs(inputs)
    res = run_bass_kernel_spmd(nc, in_maps, list(range(NCORES)))
    # per-core logits^T [VPAD, 4096 t]; t = (r, s, j); vocab sharded on cores
    parts = []
    for r in res.results:
        lg = r["logits"][:VSH]                      # [4000, 4096]
        lg = lg.reshape(VSH, NCORES, SEQ, 128).transpose(2, 1, 3, 0)
        parts.append(lg.reshape(SEQ, T, VSH))       # [s, t, 4000]
    full = np.concatenate(parts, axis=-1)           # [B, T, V]
    return np.ascontiguousarray(full, dtype=np.float32)


# revision 19
# speedup vs baseline: 1.2392x; 1.0688x over previous
"""GPT forward pass on 8 Trainium2 NeuronCores — v3.

Sharding:
  - Residual/trunk GEMMs (Wo, FFN) are token-parallel: core c owns q-tile c
    (rows 128c..128c+127) of each of the 4 sequences (512 tokens/core).
  - QKV projection + attention are head-sharded: core c owns heads {c, c+8}
    over ALL 4096 tokens. Per layer the post-LN hidden states h^T are
    AllGathered (1 MB/rank, bf16); each core computes Q/K/V for its two heads
    over all tokens and runs full causal attention for its 8 (seq, head)
    pairs, skipping above-diagonal 128x128 tiles. Attention outputs return to
    token owners via AllToAll (1 MB/rank), pre-chunked by consumer.
  - lm_head is vocab-sharded (4000 cols/core, padded to 4096).

LayerNorm gains/biases are folded into weights host-side, so on-chip LN is
just (x-mean)*rstd; transposition h -> h^T runs on the DMA xbar
(dma_start_transpose), not the PE. The V-path bias is folded into bo via
bo' = bo + (b@Wv)@Wo. The causal mask is added to scores on the PE (additive
-1e4 tile through the identity), softmax denominators come free from a
ones-column appended to V, and each (pair, kt) needs a single wide exp on
ScalarE thanks to 2-bank PSUM score tiles. Residual-add -> LN -> transpose is
fused per token tile straight after the W2/Wo accumulators close, so the
next stage's gather starts as early as possible. All matmuls bf16 with fp32
PSUM accumulate; the residual stream stays fp32.

PSUM discipline: one pool of 4 two-bank [128,1024] f32 tiles. At most one
OPEN accumulation group per 2KB bank at any time (start=True zeroes its
bank); concurrent groups always sit in different banks.
"""

import os
import sys

for _p in ("/opt/trn_rl_repo",):
    if os.path.isdir(_p) and _p not in sys.path:
        sys.path.insert(0, _p)

import numpy as np
import ml_dtypes

BF16NP = ml_dtypes.bfloat16

import concourse.bass as bass
import concourse.mybir as mybir
import concourse.tile as tile
from concourse import bacc
from concourse.bass_utils import run_bass_kernel_spmd
from concourse.masks import make_identity

F32 = mybir.dt.float32
BF = mybir.dt.bfloat16
AF = mybir.ActivationFunctionType

V, C, T, H, L, B = 32000, 1024, 1024, 16, 4, 4
HD = C // H          # 64
FF = 4 * C           # 4096
NCORES = 8
TL = 512             # local tokens per core (4 seqs x 128)
SEQ = B              # 4
NT = TL // 128       # 4
NCT = C // 128       # 8
VSH = V // NCORES    # 4000
VPAD = 4096          # padded vocab shard
LN_EPS = 1e-5

HT_SZ = C * TL            # one rank's h^T payload elems (512K)
O_SZ = NCORES * 128 * TL  # o payload elems per rank (8 consumer chunks)

# packed causal slot table: slot(kt, qt) defined for kt <= qt
_SLOT = {}
_off = 0
for _kt in range(8):
    for _qt in range(_kt, 8):
        _SLOT[(_kt, _qt)] = _off
        _off += 1
NSLOT = _off  # 36

_prog_cache = {}


def _ap(t, offset, pattern):
    return bass.AP(tensor=t.tensor if isinstance(t, bass.AP) else t, offset=offset, ap=pattern)


def _build(LL=L, debug=False, sim=False):
    key = (LL, debug, sim)
    if key in _prog_cache:
        return _prog_cache[key]

    nc = bacc.Bacc("TRN2", target_bir_lowering=False, debug=False, num_devices=NCORES)

    x0_d = nc.dram_tensor("x0", [TL, C], F32, kind="ExternalInput")
    maskD_d = nc.dram_tensor("maskD", [128, 128], BF, kind="ExternalInput")
    wq_d = nc.dram_tensor("wq", [L, C, 128], BF, kind="ExternalInput")
    wk_d = nc.dram_tensor("wk", [L, C, 128], BF, kind="ExternalInput")
    wv_d = nc.dram_tensor("wv", [L, C, 128], BF, kind="ExternalInput")
    qb_d = nc.dram_tensor("qb", [L, 128], F32, kind="ExternalInput")
    kb_d = nc.dram_tensor("kb", [L, 128], F32, kind="ExternalInput")
    wo_d = nc.dram_tensor("wo", [L, C, C], BF, kind="ExternalInput")
    bo_d = nc.dram_tensor("bo", [L, C], F32, kind="ExternalInput")
    w1_d = nc.dram_tensor("w1", [L, C, FF], BF, kind="ExternalInput")
    b1_d = nc.dram_tensor("b1", [L, FF], F32, kind="ExternalInput")
    w2_d = nc.dram_tensor("w2", [L, FF, C], BF, kind="ExternalInput")
    b2_d = nc.dram_tensor("b2", [L, C], F32, kind="ExternalInput")
    # wlm layout: [vg 4][ct 8][cp 128][vt 8 * 128 v]
    wlm_d = nc.dram_tensor("wlm", [4, NCT, 128, 1024], BF, kind="ExternalInput")
    blm_d = nc.dram_tensor("blm", [VPAD], F32, kind="ExternalInput")

    logits_d = nc.dram_tensor("logits", [VPAD, NCORES * TL], F32, kind="ExternalOutput")
    if debug:
        dbg_d = nc.dram_tensor("dbg", [LL, TL, C], F32, kind="ExternalOutput")
        dbgq_d = nc.dram_tensor("dbgq", [128, NCORES, TL], F32, kind="ExternalOutput")
        dbgk_d = nc.dram_tensor("dbgk", [128, NCORES, TL], F32, kind="ExternalOutput")
        dbgv_d = nc.dram_tensor("dbgv", [128, NCORES, SEQ, 2, HD + 1], F32, kind="ExternalOutput")
        dbgos_d = nc.dram_tensor("dbgos", [128, NCORES, SEQ, 128], F32, kind="ExternalOutput")
        dbgot_d = nc.dram_tensor("dbgot", [128, 4, 2, TL], F32, kind="ExternalOutput")

    with tile.TileContext(nc) as tc:
        import contextlib

        with contextlib.ExitStack() as ctx:
            # SBUF pools (per-partition KB in comments)
            const = ctx.enter_context(tc.tile_pool(name="const", bufs=1))      # 0.7
            xpool = ctx.enter_context(tc.tile_pool(name="x", bufs=1))          # 16
            hpool = ctx.enter_context(tc.tile_pool(name="h", bufs=2))          # 4
            tpool = ctx.enter_context(tc.tile_pool(name="hT", bufs=1))         # 8
            wqkv_pool = ctx.enter_context(tc.tile_pool(name="wqkv", bufs=1))   # 6
            big8 = ctx.enter_context(tc.tile_pool(name="big8", bufs=2))        # 16
            qkpool = ctx.enter_context(tc.tile_pool(name="qk", bufs=1))        # 16
            vpool = ctx.enter_context(tc.tile_pool(name="v", bufs=1))          # 8.3
            ptpool = ctx.enter_context(tc.tile_pool(name="pt", bufs=2))        # 18
            orawp = ctx.enter_context(tc.tile_pool(name="oraw", bufs=1))       # 8
            rbp = ctx.enter_context(tc.tile_pool(name="rb", bufs=1))           # 4
            osp = ctx.enter_context(tc.tile_pool(name="oS", bufs=1))           # 2
            otsend = ctx.enter_context(tc.tile_pool(name="otsend", bufs=1))    # 8
            otpool = ctx.enter_context(tc.tile_pool(name="oT", bufs=1))        # 8
            ugpool = ctx.enter_context(tc.tile_pool(name="ug", bufs=1))        # 32
            wlmpool = ctx.enter_context(tc.tile_pool(name="wlm", bufs=2))      # 32
            lgout = ctx.enter_context(tc.tile_pool(name="lgout", bufs=3))      # 6
            gbpool = ctx.enter_context(tc.tile_pool(name="gb", bufs=1))        # 4
            misc = ctx.enter_context(tc.tile_pool(name="misc", bufs=2))        # 1.5
            # PSUM: single pool of 4 two-bank [128,1024] f32 tiles = 8 banks
            pbig = ctx.enter_context(tc.tile_pool(name="pbig", bufs=4, space="PSUM"))
            dram = ctx.enter_context(tc.tile_pool(name="dram", bufs=1, space="DRAM"))

            ident = const.tile([128, 128], BF, name="ident")
            make_identity(nc, ident)
            eps_t = const.tile([128, 1], F32, name="eps")
            nc.vector.memset(eps_t[:], LN_EPS)
            maskD = const.tile([128, 128], BF, name="maskD")
            nc.sync.dma_start(out=maskD[:], in_=maskD_d[:])
            ones_r = const.tile([1, 128], BF, name="ones_r")
            nc.vector.memset(ones_r[:], 1.0)

            hT_loc = dram.tile([HT_SZ], BF, name="hT_loc")
            o_loc = dram.tile([O_SZ], BF, name="o_loc")
            rc_bounce = dram.tile([SEQ, 2 * NCT * 128], F32, name="rc_bounce")

            x_t = [xpool.tile([128, C], F32, tag=f"x{tt}", name=f"x{tt}") for tt in range(NT)]

            def bcast_row(dst, src_1d, offset, n):
                src = _ap(src_1d, offset, [[0, dst.shape[0]], [1, n]])
                nc.gpsimd.dma_start(out=dst[:], in_=src)

            def ln_tile(tt, hT):
                """(x_tt - mean) * rstd -> bf16 -> DMA-transpose into hT."""
                stats = misc.tile([128, 2, 6], F32, name="stats", tag="stats")
                xv = x_t[tt][:].rearrange("p (s d) -> p s d", s=2)
                nc.vector.bn_stats(out=stats[:, 0, :], in_=xv[:, 0, :])
                nc.vector.bn_stats(out=stats[:, 1, :], in_=xv[:, 1, :])
                mv = misc.tile([128, 2], F32, name="mv", tag="mv")
                nc.vector.bn_aggr(out=mv[:], in_=stats[:])
                rstd = misc.tile([128, 1], F32, name="rstd", tag="rstd")
                nc.scalar.activation(rstd[:], mv[:, 1:2], AF.Sqrt, bias=eps_t[:])
                nc.vector.reciprocal(rstd[:], rstd[:])
                h = hpool.tile([128, C], BF, tag="h", name=f"h{tt}")
                nc.vector.tensor_scalar(
                    out=h[:], in0=x_t[tt][:], scalar1=mv[:, 0:1], scalar2=rstd[:],
                    op0=mybir.AluOpType.subtract, op1=mybir.AluOpType.mult,
                )
                nc.sync.dma_start_transpose(out=hT[:, :, tt, :], in_=h[:])

            def new_hT():
                # hT layout [128 cp, ct, tt, 128 j]
                return tpool.tile([128, NCT, NT, 128], BF, tag="hT", name="hT")

            def gather_hT(hT, lname):
                """write local h^T -> DRAM, AllGather -> [r][ct][cp][512]."""
                nc.sync.dma_start(
                    out=_ap(hT_loc, 0, [[TL, 128], [128 * TL, NCT], [1, TL]]),
                    in_=hT[:],
                )
                full = dram.tile([NCORES * HT_SZ], BF,
                                 addr_space="Local" if sim else "Shared", name=lname)
                if sim:
                    nc.sync.dma_start(
                        out=_ap(full, 0, [[2048, HT_SZ // 2048], [1, 2048]]),
                        in_=_ap(hT_loc, 0, [[2048, HT_SZ // 2048], [1, 2048]]),
                    )
                else:
                    nc.gpsimd.collective_compute(
                        "AllGather",
                        mybir.AluOpType.bypass,
                        replica_groups=[list(range(NCORES))],
                        ins=[_ap(hT_loc, 0, [[2048, HT_SZ // 2048], [1, 2048]])],
                        outs=[_ap(full, 0, [[2048, NCORES * HT_SZ // 2048], [1, 2048]])],
                    )
                return full

            # v_sb: [128 j, kt 8, s 4, hs 2, 65]; col 64 is the ones column
            v_sb = vpool.tile([128, NCORES, SEQ, 2, HD + 1], BF, name="v_sb")
            nc.vector.memset(v_sb[:, :, :, :, HD:HD + 1], 1.0)

            # ---- x0 load, LN1(layer 0), gather ----
            hT = new_hT()
            for tt in range(NT):
                nc.sync.dma_start(out=x_t[tt][:], in_=x0_d[tt * 128:(tt + 1) * 128, :])
                ln_tile(tt, hT)
            hfull = gather_hT(hT, "hfull0")

            for l in range(LL):
                lw = l % L
                wq_sb = wqkv_pool.tile([128, NCT, 128], BF, tag="wq", name="wq_sb")
                nc.sync.dma_start(
                    out=wq_sb[:],
                    in_=_ap(wq_d, lw * C * 128, [[128, 128], [128 * 128, NCT], [1, 128]]))
                wk_sb = wqkv_pool.tile([128, NCT, 128], BF, tag="wk", name="wk_sb")
                nc.sync.dma_start(
                    out=wk_sb[:],
                    in_=_ap(wk_d, lw * C * 128, [[128, 128], [128 * 128, NCT], [1, 128]]))
                wv_sb = wqkv_pool.tile([128, NCT, 128], BF, tag="wv", name="wv_sb")
                nc.sync.dma_start(
                    out=wv_sb[:],
                    in_=_ap(wv_d, lw * C * 128, [[128, 128], [128 * 128, NCT], [1, 128]]))
                qb_t = misc.tile([128, 1], F32, tag="qb", name="qb_t")
                nc.sync.dma_start(out=qb_t[:], in_=_ap(qb_d, lw * 128, [[1, 128], [1, 1]]))
                kb_t = misc.tile([128, 1], F32, tag="kb", name="kb_t")
                nc.sync.dma_start(out=kb_t[:], in_=_ap(kb_d, lw * 128, [[1, 128], [1, 1]]))

                # ---- QKV for own heads over ALL tokens ----
                qT_sb = qkpool.tile([128, NCORES, TL], BF, tag="qT", name="qT_sb")
                kT_sb = qkpool.tile([128, NCORES, TL], BF, tag="kT", name="kT_sb")
                for r in range(NCORES):
                    ch = big8.tile([128, NCT, 512], BF, tag="big8", name="hch")
                    if r == 0:
                        # split first chunk so the first matmul starts early
                        nc.sync.dma_start(
                            out=ch[:, 0, :],
                            in_=_ap(hfull, r * HT_SZ, [[TL, 128], [1, TL]]))
                        nc.sync.dma_start(
                            out=ch[:, 1:NCT, :],
                            in_=_ap(hfull, r * HT_SZ + 128 * TL,
                                    [[TL, 128], [128 * TL, NCT - 1], [1, TL]]))
                    else:
                        nc.sync.dma_start(
                            out=ch[:],
                            in_=_ap(hfull, r * HT_SZ, [[TL, 128], [128 * TL, NCT], [1, TL]]))
                    tqk = pbig.tile([128, 1024], F32, tag="big", name="tqk")
                    psK = tqk[:, 0:512]
                    psQ = tqk[:, 512:1024]
                    for ct in range(NCT):
                        nc.tensor.matmul(psK, wk_sb[:, ct, :], ch[:, ct, :],
                                         start=(ct == 0), stop=(ct == NCT - 1))
                        nc.tensor.matmul(psQ, wq_sb[:, ct, :], ch[:, ct, :],
                                         start=(ct == 0), stop=(ct == NCT - 1))
                    nc.scalar.activation(kT_sb[:, r, :], psK, AF.Identity, bias=kb_t[:])
                    nc.scalar.activation(qT_sb[:, r, :], psQ, AF.Identity, bias=qb_t[:])
                    # V groups sequential, alternating banks of one tile
                    for s in range(SEQ):
                        tv = pbig.tile([128, 1024], F32, tag="big", name="tv")
                        psV = tv[:, 0:128]
                        for ct in range(NCT):
                            nc.tensor.matmul(
                                psV, ch[:, ct, s * 128:(s + 1) * 128], wv_sb[:, ct, :],
                                start=(ct == 0), stop=(ct == NCT - 1))
                        nc.vector.tensor_copy(
                            out=v_sb[:, r, s, :, 0:HD],
                            in_=psV.rearrange("p (h d) -> p h d", h=2))

                if debug and l == 0:
                    nc.gpsimd.dma_start(out=dbgq_d[:], in_=qT_sb[:])
                    nc.gpsimd.dma_start(out=dbgk_d[:], in_=kT_sb[:])
                    nc.gpsimd.dma_start(out=dbgv_d[:], in_=v_sb[:])

                # ---- causal attention, 8 (s, hs) pairs ----
                # one 2-bank score tile per (pair, kt): cols [0:(8-kt)*128]
                # contiguous across the bank boundary; diag mask added on PE;
                # ONE exp per kt covers all its q tiles.
                oT_send = otsend.tile([128, NCORES, SEQ, 128], BF, tag="ots", name="oT_send")
                for s in range(SEQ):
                    oraw = orawp.tile([HD + 1, 2, NCORES, 128], F32, tag="oraw", name="oraw")
                    pTs = {}
                    for hs in range(2):
                        p0 = hs * HD
                        pT = ptpool.tile([128, NSLOT, 128], BF, tag="pt", name="pT")
                        pTs[hs] = pT
                        for kt in range(NCORES):
                            wtot = (NCORES - kt) * 128
                            tst = pbig.tile([128, 1024], F32, tag="big", name="tst")
                            if kt < 4:
                                nc.tensor.matmul(
                                    tst[:, 0:512],
                                    kT_sb[p0:p0 + HD, kt, s * 128:(s + 1) * 128],
                                    qT_sb[p0:p0 + HD, kt:kt + 4, s * 128:(s + 1) * 128],
                                    start=True, stop=False)
                                nc.tensor.matmul(
                                    tst[:, 0:128], maskD[:], ident[:],
                                    start=False, stop=True)
                                nc.tensor.matmul(
                                    tst[:, 512:wtot],
                                    kT_sb[p0:p0 + HD, kt, s * 128:(s + 1) * 128],
                                    qT_sb[p0:p0 + HD, kt + 4:NCORES, s * 128:(s + 1) * 128],
                                    start=True, stop=True)
                            else:
                                nc.tensor.matmul(
                                    tst[:, 0:wtot],
                                    kT_sb[p0:p0 + HD, kt, s * 128:(s + 1) * 128],
                                    qT_sb[p0:p0 + HD, kt:NCORES, s * 128:(s + 1) * 128],
                                    start=True, stop=False)
                                nc.tensor.matmul(
                                    tst[:, 0:128], maskD[:], ident[:],
                                    start=False, stop=True)
                            sl = _SLOT[(kt, kt)]
                            nc.scalar.activation(
                                pT[:, sl:sl + NCORES - kt, :], tst[:, 0:wtot], AF.Exp)
                    for hs in range(2):
                        pT = pTs[hs]
                        for qt in range(NCORES):
                            tov = pbig.tile([128, 1024], F32, tag="big", name="tov")
                            ov = tov[0:HD + 1, 0:128]
                            for kt in range(qt + 1):
                                sl = _SLOT[(kt, qt)]
                                nc.tensor.matmul(
                                    ov, v_sb[:, kt, s, hs, :], pT[:, sl, :],
                                    start=(kt == 0), stop=(kt == qt))
                            nc.vector.tensor_copy(out=oraw[:, hs, qt, :], in_=ov)
                    # denominators -> reciprocal in place -> bounce-broadcast
                    nc.vector.reciprocal(oraw[HD:HD + 1, :, :, :], oraw[HD:HD + 1, :, :, :])
                    nc.sync.dma_start(out=rc_bounce[s, :], in_=oraw[HD:HD + 1, :, :, :])
                    rb = rbp.tile([HD, 2, NCORES, 128], BF, tag="rb", name="rb")
                    nc.gpsimd.dma_start(
                        out=rb[:], in_=_ap(rc_bounce, s * 2 * NCORES * 128,
                                           [[0, HD], [1, 2 * NCORES * 128]]))
                    oS = osp.tile([HD, NCORES, 128], BF, tag="oS", name="oS")
                    for hs in range(2):
                        for qt in range(NCORES):
                            dst = oT_send[0:HD, qt, s, :] if hs == 0 else oS[:, qt, :]
                            nc.vector.tensor_mul(
                                out=dst, in0=oraw[0:HD, hs, qt, :], in1=rb[:, hs, qt, :])
                    nc.sync.dma_start(out=oT_send[HD:128, :, s, :], in_=oS[:])
                    nc.sync.dma_start(
                        out=_ap(o_loc, s * 128, [[TL, 128], [128 * TL, NCORES], [1, 128]]),
                        in_=oT_send[:, :, s, :])

                if debug and l == 0:
                    nc.gpsimd.dma_start(out=dbgos_d[:], in_=oT_send[:])

                # prefetch Wo weight halves while the exchange is in flight
                wos_t = []
                for nf in range(2):
                    wos = big8.tile([128, NCT, 512], BF, tag="big8", name="wos")
                    nc.sync.dma_start(
                        out=wos[:],
                        in_=_ap(wo_d, lw * C * C + nf * 512,
                                [[C, 128], [128 * C, NCT], [1, 512]]))
                    wos_t.append(wos)
                bo_row = gbpool.tile([1, C], BF, tag="bo", name="bo_row")
                nc.gpsimd.dma_start(out=bo_row[:], in_=_ap(bo_d, lw * C, [[0, 1], [1, C]]))

                # ---- O exchange: AllToAll, blocks pre-chunked by consumer ----
                o_x = dram.tile([O_SZ], BF, addr_space="Local", name=f"ox{l}")
                if sim:
                    nc.sync.dma_start(
                        out=_ap(o_x, 0, [[2048, O_SZ // 2048], [1, 2048]]),
                        in_=_ap(o_loc, 0, [[2048, O_SZ // 2048], [1, 2048]]))
                else:
                    nc.gpsimd.collective_compute(
                        "AllToAll",
                        mybir.AluOpType.bypass,
                        replica_groups=[list(range(NCORES))],
                        ins=[_ap(o_loc, 0, [[2048, O_SZ // 2048], [1, 2048]])],
                        outs=[_ap(o_x, 0, [[2048, O_SZ // 2048], [1, 2048]])])

                # assemble oT [128 p, cc 4, g 2, 512 t'] (channel ct = g*4+cc):
                # block r rows 0-63 = head r (g=0), rows 64-127 = head r+8 (g=1)
                oT = otpool.tile([128, 4, 2, TL], BF, tag="oT", name="oT")
                for r in range(NCORES):
                    nc.sync.dma_start(
                        out=oT[(r % 2) * HD:(r % 2) * HD + HD, r // 2, :, :],
                        in_=_ap(o_x, r * 128 * TL, [[TL, HD], [HD * TL, 2], [1, TL]]))

                if debug and l == 0:
                    nc.gpsimd.dma_start(out=dbgot_d[:], in_=oT[:])

                # ---- Wo + residual; nf-major, fused per-tile LN2 on nf1 ----
                h2T = new_hT()
                for nf in range(2):
                    pwo = [pbig.tile([128, 1024], F32, tag="big", name="pwo")
                           for _ in range(2)]
                    for ct in range(NCT):
                        for tt in range(NT):
                            nc.tensor.matmul(
                                pwo[tt // 2][:, (tt % 2) * 512:(tt % 2 + 1) * 512],
                                oT[:, ct % 4, ct // 4, tt * 128:(tt + 1) * 128],
                                wos_t[nf][:, ct, :],
                                start=(ct == 0), stop=False)
                    for tt in range(NT):
                        sl = pwo[tt // 2][:, (tt % 2) * 512:(tt % 2 + 1) * 512]
                        nc.tensor.matmul(
                            sl[0:128, :], ones_r[:], bo_row[0:1, nf * 512:(nf + 1) * 512],
                            start=False, stop=True)
                        nc.vector.tensor_add(
                            out=x_t[tt][:, nf * 512:(nf + 1) * 512],
                            in0=x_t[tt][:, nf * 512:(nf + 1) * 512], in1=sl)
                        if nf == 1:
                            ln_tile(tt, h2T)

                # ---- FFN ----
                b1_t = misc.tile([128, FF // 128], F32, tag="b1", name="b1_t")
                nc.gpsimd.dma_start(
                    out=b1_t[:], in_=_ap(b1_d, lw * FF, [[1, 128], [128, FF // 128]]))
                b2_row = gbpool.tile([1, C], BF, tag="b2", name="b2_row")
                nc.gpsimd.dma_start(out=b2_row[:], in_=_ap(b2_d, lw * C, [[0, 1], [1, C]]))
                ug = ugpool.tile([128, FF // 128, 512], BF, tag="ug", name="ug")
                for fg in range(8):
                    w1s = big8.tile([128, NCT, 512], BF, tag="big8", name="w1s")
                    nc.sync.dma_start(
                        out=w1s[:],
                        in_=_ap(w1_d, lw * C * FF + fg * 512,
                                [[FF, 128], [128 * FF, NCT], [1, 512]]))
                    pf = [pbig.tile([128, 1024], F32, tag="big", name="pf") for _ in range(2)]
                    for ct in range(NCT):
                        for f4 in range(4):
                            nc.tensor.matmul(
                                pf[f4 // 2][:, (f4 % 2) * 512:(f4 % 2 + 1) * 512],
                                w1s[:, ct, f4 * 128:(f4 + 1) * 128],
                                h2T[:, ct, :, :],
                                start=(ct == 0), stop=(ct == NCT - 1))
                    for f4 in range(4):
                        ft = fg * 4 + f4
                        nc.scalar.activation(
                            ug[:, ft, :], pf[f4 // 2][:, (f4 % 2) * 512:(f4 % 2 + 1) * 512],
                            AF.Gelu, bias=b1_t[:, ft:ft + 1])

                # ---- W2 + residual; nf-major, fused per-tile LN1(l+1) on nf1 ----
                hT = new_hT()
                for nf in range(2):
                    p2 = [pbig.tile([128, 1024], F32, tag="big", name="p2")
                          for _ in range(2)]
                    for ftg in range(4):
                        w2s = big8.tile([128, 8, 512], BF, tag="big8", name="w2s")
                        nc.sync.dma_start(
                            out=w2s[:],
                            in_=_ap(w2_d, lw * FF * C + ftg * 8 * 128 * C + nf * 512,
                                    [[C, 128], [128 * C, 8], [1, 512]]))
                        for f8 in range(8):
                            ft = ftg * 8 + f8
                            for tt in range(NT):
                                nc.tensor.matmul(
                                    p2[tt // 2][:, (tt % 2) * 512:(tt % 2 + 1) * 512],
                                    ug[:, ft, tt * 128:(tt + 1) * 128],
                                    w2s[:, f8, :],
                                    start=(ft == 0), stop=False)
                    for tt in range(NT):
                        sl = p2[tt // 2][:, (tt % 2) * 512:(tt % 2 + 1) * 512]
                        nc.tensor.matmul(
                            sl[0:128, :], ones_r[:], b2_row[0:1, nf * 512:(nf + 1) * 512],
                            start=False, stop=True)
                        nc.vector.tensor_add(
                            out=x_t[tt][:, nf * 512:(nf + 1) * 512],
                            in0=x_t[tt][:, nf * 512:(nf + 1) * 512], in1=sl)
                        if nf == 1:
                            if debug:
                                nc.sync.dma_start(
                                    out=dbg_d[l, tt * 128:(tt + 1) * 128, :], in_=x_t[tt][:])
                            ln_tile(tt, hT)
                hfull = gather_hT(hT, f"hfull{l + 1}")

            # ---- lm_head over the final gather (hfull = LN_f(x) gathered) ----
            blm_sb = misc.tile([128, 32], F32, tag="blm", name="blm_sb")
            nc.sync.dma_start(out=blm_sb[:], in_=_ap(blm_d, 0, [[1, 128], [128, 32]]))
            wlm_tiles = {}

            def load_wlm(vg):
                t = wlmpool.tile([128, NCT, 8, 128], BF, tag="wlm", name="wlm_sb")
                nc.sync.dma_start(
                    out=t[:],
                    in_=_ap(wlm_d, vg * NCT * 128 * 1024,
                            [[1024, 128], [128 * 1024, NCT], [1, 1024]]))
                wlm_tiles[vg] = t

            load_wlm(0)
            for vg in range(4):
                wlm_sb = wlm_tiles.pop(vg)
                for tg in range(NCORES):
                    if tg == 1 and vg < 3:
                        load_wlm(vg + 1)
                    hfch = big8.tile([128, NCT, 512], BF, tag="big8", name="hfch")
                    nc.sync.dma_start(
                        out=hfch[:],
                        in_=_ap(hfull, tg * HT_SZ, [[TL, 128], [128 * TL, NCT], [1, TL]]))
                    for vt in range(8):
                        plm = pbig.tile([128, 1024], F32, tag="big", name="plm")
                        ps = plm[:, 0:512]
                        for ct in range(NCT):
                            nc.tensor.matmul(
                                ps, wlm_sb[:, ct, vt, :], hfch[:, ct, :],
                                start=(ct == 0), stop=(ct == NCT - 1))
                        lg = lgout.tile([128, 512], F32, tag="lg", name="lg")
                        nc.scalar.activation(
                            lg[:], ps, AF.Identity,
                            bias=blm_sb[:, vg * 8 + vt:vg * 8 + vt + 1])
                        row0 = (vg * 8 + vt) * 128
                        nc.sync.dma_start(
                            out=logits_d[row0:row0 + 128, tg * 512:(tg + 1) * 512],
                            in_=lg[:])

    nc.compile()
    _prog_cache[key] = nc
    return nc


def _prep_inputs(inputs):
    f = {k: np.asarray(v) for k, v in inputs.items()}
    idx = f["idx"].astype(np.int64)
    emb = f["emb"].astype(np.float32)
    pos = f["pos_enc"].astype(np.float32)
    x_full = emb[idx] + pos[None, :, :]          # [B, T, C] f32

    scale = HD ** -0.5
    bf = lambda a: np.ascontiguousarray(a, dtype=np.float32).astype(BF16NP)
    f32 = lambda a: np.ascontiguousarray(a, dtype=np.float32)

    Wq, Wk, Wv, Wo = (f[k].astype(np.float32) for k in ("Wq", "Wk", "Wv", "Wo"))
    W1, W2 = f["W1"].astype(np.float32), f["W2"].astype(np.float32)
    g1, b1n = f["ln1_g"].astype(np.float32), f["ln1_b"].astype(np.float32)
    g2, b2n = f["ln2_g"].astype(np.float32), f["ln2_b"].astype(np.float32)
    gf, bfn = f["lnf_g"].astype(np.float32), f["lnf_b"].astype(np.float32)
    bo, b1, b2 = (f[k].astype(np.float32) for k in ("bo", "b1", "b2"))
    Wlm = f["Wlm"].astype(np.float32)
    blm = f["blm"].astype(np.float32)

    # fold LN gains into weight rows, LN biases into per-channel output biases
    wqf = g1[:, :, None] * Wq * scale            # [L, C, C]
    wkf = g1[:, :, None] * Wk
    wvf = g1[:, :, None] * Wv
    qb_full = np.einsum("lc,lcd->ld", b1n, Wq) * scale   # [L, C]
    kb_full = np.einsum("lc,lcd->ld", b1n, Wk)
    vb_full = np.einsum("lc,lcd->ld", b1n, Wv)
    bo_eff = bo + np.einsum("lc,lcd->ld", vb_full, Wo)   # V bias folded thru Wo
    w1f = g2[:, :, None] * W1
    b1_eff = b1 + np.einsum("lc,lcf->lf", b2n, W1)
    wlmf = gf[:, None] * Wlm                     # [C, V]
    blm_eff = blm + bfn @ Wlm                    # [V]

    # additive causal mask for diagonal tiles, supplied as lhsT = M^T where
    # M[k, q] = 0 if k <= q else -1e4 (exp -> 0); M^T[q, k] = M[k, q]
    kk = np.arange(128)[None, :]
    qq = np.arange(128)[:, None]
    maskD = np.where(kk <= qq, 0.0, -1e4).astype(np.float32).astype(BF16NP)

    in_maps = []
    for c in range(NCORES):
        cols = np.r_[c * HD:(c + 1) * HD, (c + 8) * HD:(c + 9) * HD]
        x0_c = np.ascontiguousarray(
            x_full[:, 128 * c:128 * (c + 1), :].reshape(TL, C), dtype=np.float32)

        # lm shard, folded + padded to 4096, tiled [vg][ct][cp][vt*128]
        wlm_c = wlmf[:, c * VSH:(c + 1) * VSH]
        wlm_pad = np.zeros((C, VPAD), np.float32)
        wlm_pad[:, :VSH] = wlm_c
        wlm_t = wlm_pad.reshape(C, 4, 1024).transpose(1, 0, 2)     # [vg, C, 1024]
        wlm_t = wlm_t.reshape(4, NCT, 128, 1024)
        blm_pad = np.zeros((VPAD,), np.float32)
        blm_pad[:VSH] = blm_eff[c * VSH:(c + 1) * VSH]

        im = {
            "x0": x0_c,
            "maskD": maskD,
            "wq": bf(wqf[:, :, cols]),
            "wk": bf(wkf[:, :, cols]),
            "wv": bf(wvf[:, :, cols]),
            "qb": f32(qb_full[:, cols]),
            "kb": f32(kb_full[:, cols]),
            "wo": bf(Wo),
            "bo": f32(bo_eff),
            "w1": bf(w1f),
            "b1": f32(b1_eff),
            "w2": bf(W2),
            "b2": f32(b2),
            "wlm": np.ascontiguousarray(wlm_t).astype(BF16NP),
            "blm": f32(blm_pad),
        }
        in_maps.append(im)
    return in_maps


def kernel(**inputs):
    nc = _build()
    in_maps = _prep_inputs(inputs)
    res = run_bass_kernel_spmd(nc, in_maps, list(range(NCORES)))
    # per-core logits^T [VPAD, 4096 t]; t = (r, s, j); vocab sharded on cores
    parts = []
    for r in res.results:
        lg = r["logits"][:VSH]                      # [4000, 4096]
        lg = lg.reshape(VSH, NCORES, SEQ, 128).transpose(2, 1, 3, 0)
        parts.append(lg.reshape(SEQ, T, VSH))       # [s, t, 4000]
    full = np.concatenate(parts, axis=-1)           # [B, T, V]
    return np.ascontiguousarray(full, dtype=np.float32)
